# revision 63
# baseline (speedup 1.0000x reference)
"""Trainium2 Bass kernel for nn_AttentionAggregator3d.

Math (per batch b):
    zmf = zm.reshape(CM, N)                     # N = D*W*H = 4096 tokens
    q = Wq @ zmf + bq ; k = Wk @ zmf + bk       # (16, N)
    v = Wv @ zmf + bv                           # (128, N)
    A = softmax_n(q^T k)                        # (N, N), softmax over keys n
    out = v @ A^T ; result = zc + gamma * out

Kernel structure (ScalarE-stream-bound design, ~58-60us/core):
  * logits^T[n, m] = k_n . q_m with G = Wk^T Wq folded on host; the
    query-side transform tq = G^T zm_q (128 x 1024) is computed ONCE and
    used as the bf16 MOVING operand of every logits matmul, with the key
    chunk zm_j as the stationary.  (The old version materialized t = G zm
    over all 4096 keys and burned 8 ScalarE copies staging it; ScalarE is
    the bottleneck engine, so those copies came straight out of the
    critical path.)
  * All matmul operands are bf16 (host pre-converts zm/G/Wv^T); exp output
    E is bf16 too, which halves SBUF traffic and lets the DVE denominator
    accumulation run in the 16-bit DVE perf modes.
  * Steady state: ScalarE streams 32 back-to-back [128,1024] exps
    (~1.05us each = the roofline); TensorE (~95% busy) does per chunk two
    512-wide logits matmuls, two 512-wide PV accumulations and an
    amortized 1/4 vproj batch; DVE accumulates both denominator halves
    except 8 early-chunk halves on GPSIMD.
  * Tail: three ones-matmul folds -> fast DVE reciprocal (straight from
    PSUM) -> GPSIMD partition_broadcast -> DVE multiply + fused
    multiply-add (gamma and gamma*bv are folded host-side into a single
    scale and into zc, so gamma=0 / negative gamma need no special path).
  * Sharding: 8 cores = batch (2) x query-block (4, 1024 queries each),
    zm rolled per core so its query block sits at columns 0:1024.
  * ACT tables are prefetched with a dummy exp at t=0 so the table load
    overlaps the input DMAs; the zm[:, 0:1024] DMA (which gates the first
    exp) is split across two queues because transfers serialize per queue.
"""

import os
import sys
import types

import ml_dtypes
import numpy as np

import concourse.bacc as bacc_mod
import concourse.tile as tile
from concourse import mybir
from concourse.bass_utils import run_bass_kernel_spmd

B, CC, CM, P = 2, 128, 128, 16
N = 16 * 16 * 16          # 4096 tokens
MBLK = N // 4             # 1024 queries per core
NCORES = 8
NCHUNK = N // 128         # 32 key chunks of 128

F32 = mybir.dt.float32
F32R = mybir.dt.float32r
BF16 = mybir.dt.bfloat16
AF = mybir.ActivationFunctionType
ALU = mybir.AluOpType

LAST_RESULTS = None  # BassKernelResults of the most recent run (for test.py)


def _ensure_ntff_hook() -> bool:
    """The grading image lacks antenv.axon_hooks; synthesize it from the
    boot module's ctypes NTFF driver so trace=True works under axon."""
    try:
        import antenv.axon_hooks  # noqa: F401

        return True
    except ImportError:
        pass
    try:
        import antenv
        from trn_agent_boot.trn_boot import _ntff_profile_via_ctypes

        hook = _ntff_profile_via_ctypes("/opt/axon/libaxon_pjrt.so")
        mod = types.ModuleType("antenv.axon_hooks")
        mod.get_axon_ntff_profile_hook = lambda: hook
        mod.set_axon_ntff_profile_hook = lambda h: None
        sys.modules["antenv.axon_hooks"] = mod
        antenv.axon_hooks = mod
        return hook is not None
    except Exception:
        return False


# Route Exp and Ln to the one table set that holds both, so the kernel pays a
# single ACT_TABLE_LOAD instead of three (exp -> ln -> exp again).
_orig_gat = bacc_mod.get_activation_tables
_COMBINED_SET = "natural_log_exp_and_others"


def _patched_gat(arch):
    tabs = _orig_gat(arch)
    if _COMBINED_SET in tabs:
        for name, fns in tabs.items():
            if name != _COMBINED_SET:
                fns.discard(AF.Exp)
                fns.discard(AF.Ln)
    return tabs


bacc_mod.get_activation_tables = _patched_gat


def _build(use_qk_bias: bool):
    nc = bacc_mod.Bacc(
        "TRN2",
        target_bir_lowering=False,
        debug=False,
        num_devices=NCORES,
    )

    zm_d = nc.dram_tensor("zm", (CM, N), BF16, kind="ExternalInput").ap()
    zc_d = nc.dram_tensor("zc", (CC, MBLK), F32, kind="ExternalInput").ap()
    gt_d = nc.dram_tensor("gt", (CM, CM), BF16, kind="ExternalInput").ap()
    wvt_d = nc.dram_tensor("wvt", (CM, CC), BF16, kind="ExternalInput").ap()
    # packed per-partition scalars: col 0 = gamma (gamma*bv is pre-added
    # into zc on the host)
    sc_d = nc.dram_tensor("sc", (CC, 1), F32, kind="ExternalInput").ap()
    onesc_d = nc.dram_tensor("onesc", (128, 1), BF16, kind="ExternalInput").ap()
    if use_qk_bias:
        u_d = nc.dram_tensor("u", (CM, 1), BF16, kind="ExternalInput").ap()
    out_d = nc.dram_tensor("out", (CC, MBLK), F32, kind="ExternalOutput").ap()

    # denominator routing (see module docstring): half 0 accumulates on the
    # DVE (acc0); half 1 goes to GPSIMD for early chunks (it is slow per
    # add, so keep it off the final-chunk critical path), else to the DVE
    # acc accumulator.  PE is the stream pacer at ~95%+ busy, so it gets no
    # ones-matmuls during the stream; the cross-partition folds happen in
    # three matmuls after the last exp.
    GP_H1 = [j for j in range(NCHUNK) if j % 3 == 1 and j < 24]

    with tile.TileContext(nc) as tc:
        with (
            tc.tile_pool(name="consts", bufs=1) as consts,
            tc.tile_pool(name="epool", bufs=10) as epool,
            tc.tile_pool(name="lpool", bufs=2, space="PSUM") as lpool,
            tc.tile_pool(name="stage", bufs=1, space="PSUM") as stage,
            tc.tile_pool(name="opool", bufs=1, space="PSUM") as opool,
            tc.tile_pool(name="spool", bufs=1, space="PSUM") as spool,
        ):
            zm_bf = consts.tile([CM, N], BF16, tag="zm")
            tq_bf = consts.tile([CM, MBLK], BF16, tag="tq")
            vt_bf = consts.tile([128, N], BF16, tag="vt")  # chunk j at cols 128j
            zc_sb = consts.tile([CC, MBLK], F32, tag="zc")
            gt_sb = consts.tile([CM, CM], BF16, tag="gt")
            wvt_sb = consts.tile([CM, CC], BF16, tag="wvt")
            sc_sb = consts.tile([CC, 1], F32, tag="sc")
            ones_col = consts.tile([128, 1], BF16, tag="onesc")
            acc0 = consts.tile([128, 512], BF16, tag="acc0")   # DVE, half 0
            acc = consts.tile([128, 512], BF16, tag="acc")     # DVE, half 1
            accg = consts.tile([128, 512], BF16, tag="accg")   # GPSIMD, half 1
            rvec = consts.tile([1, MBLK], F32, tag="rvec")
            rb_sb = consts.tile([128, MBLK], F32, tag="rb")
            tmp_sb = consts.tile([CC, MBLK], F32, tag="tmp")
            out_sb = consts.tile([CC, MBLK], F32, tag="outsb")
            scr = consts.tile([1, 1], F32, tag="scr")
            if use_qk_bias:
                u_sb = consts.tile([CM, 1], BF16, tag="u")
                rn_sb = consts.tile([128, NCHUNK], F32, tag="rn")

            # (A PE warm-up via junk matmuls during the DMA wait was tried
            # and removed: the HAM un-throttle lags the busy burst by ~2us,
            # so the dummies delay the first real matmul without making the
            # tq/logits-0 chain run warm.)

            # ---- input DMAs.  The critical chain is zm[:, 0:1024] + gt ->
            # tq -> cast -> logits 0 -> exp 0.  DMA *transfers* serialize per
            # issuing queue (~2.5us per 1024 bf16 columns), so the two tq
            # halves of zm go on different queues (sync / scalar) and the
            # rest follows on gpsimd, ordered by first use. ----
            nc.sync.dma_start(zm_bf[:, 0:256], zm_d[:, 0:256])
            nc.scalar.dma_start(gt_sb[:], gt_d)
            nc.sync.dma_start(zm_bf[:, 256:512], zm_d[:, 256:512])
            nc.scalar.dma_start(zm_bf[:, 512:768], zm_d[:, 512:768])
            nc.scalar.dma_start(zm_bf[:, 768:1024], zm_d[:, 768:1024])

            # ACT table prefetch: a dummy exp with no real consumers makes
            # ScalarE pay the ~2.7us exp/ln table load during the input DMAs
            # instead of in front of the first real exp
            nc.scalar.activation(scr[0:1, 0:1], gt_sb[0:1, 0:1], AF.Exp)

            nc.sync.dma_start(ones_col[:], onesc_d)
            nc.gpsimd.dma_start(wvt_sb[:], wvt_d)
            if use_qk_bias:
                nc.gpsimd.dma_start(u_sb[:], u_d)
            nc.gpsimd.dma_start(zm_bf[:, 1024:2048], zm_d[:, 1024:2048])
            nc.gpsimd.dma_start(zm_bf[:, 2048:3072], zm_d[:, 2048:3072])
            nc.gpsimd.dma_start(zm_bf[:, 3072:4096], zm_d[:, 3072:4096])
            nc.gpsimd.dma_start(sc_sb[:], sc_d)
            nc.sync.dma_start(zc_sb[:], zc_d)

            gam_ap = sc_sb[:, 0:1]

            out_ps = opool.tile([CC, MBLK], F32, tag="out")
            s_ps = spool.tile([1, 512], F32, tag="s")  # half-0 denom fold

            # ---- tq = G zm_q over the core's 1024 query columns, in four
            # 256-column stages so each DVE cast overlaps the next matmul
            # and each stage starts as soon as its zm DMA piece lands ----
            tq_ps = lpool.tile([128, MBLK], F32, tag="L")
            for q in range(4):
                qs = slice(q * 256, (q + 1) * 256)
                nc.tensor.matmul(
                    tq_ps[:, qs], gt_sb[:], zm_bf[:, qs], start=True, stop=True
                )
                nc.vector.tensor_copy(tq_bf[:, qs], tq_ps[:, qs])

            def emit_vt_batch(i):
                # vt chunk j = (zm chunk j)^T @ Wv^T for j in 4i..4i+3
                vps = stage.tile([128, 512], F32, tag="S")
                for k in range(4):
                    j = 4 * i + k
                    nc.tensor.matmul(
                        vps[:, 128 * k : 128 * (k + 1)],
                        zm_bf[:, 128 * j : 128 * (j + 1)],
                        wvt_sb[:],
                        start=True,
                        stop=True,
                    )
                nc.vector.tensor_copy(vt_bf[:, i * 512 : (i + 1) * 512], vps[:])
                if use_qk_bias:
                    rnps = stage.tile([128, 512], F32, tag="S")
                    for k in range(4):
                        j = 4 * i + k
                        nc.tensor.matmul(
                            rnps[:, k : k + 1],
                            zm_bf[:, 128 * j : 128 * (j + 1)],
                            u_sb[:],
                            start=True,
                            stop=True,
                        )
                    nc.vector.tensor_copy(
                        rn_sb[:, 4 * i : 4 * (i + 1)], rnps[:, 0:4]
                    )

            if use_qk_bias:
                # the chunk-0..3 exp biases must be materialized before the
                # first exp reads them
                emit_vt_batch(0)

            e_tiles = {}

            LAG = int(os.environ.get("BASS_PV_LAG", "2"))
            for j in range(NCHUNK + LAG):
                if j < NCHUNK:
                    if j % 4 == 2 and j // 4 + 1 <= 7:
                        emit_vt_batch(j // 4 + 1)
                    # logits^T chunk j: (keys 128, queries 1024)
                    lps = lpool.tile([128, MBLK], F32, tag="L")
                    for h in range(2):
                        nc.tensor.matmul(
                            lps[:, h * 512 : (h + 1) * 512],
                            zm_bf[:, 128 * j : 128 * (j + 1)],
                            tq_bf[:, h * 512 : (h + 1) * 512],
                            start=True,
                            stop=True,
                        )
                    ej = epool.tile([128, MBLK], BF16, tag="E")
                    bias = rn_sb[:, j : j + 1] if use_qk_bias else 0.0
                    nc.scalar.activation(ej[:], lps[:], AF.Exp, bias=bias)
                    e_tiles[j] = ej
                    if j == 1 and not use_qk_bias:
                        emit_vt_batch(0)
                    # softmax-denominator accumulation for chunk j
                    if j == 0:
                        nc.vector.tensor_copy(acc0[:], ej[:, 0:512])
                    else:
                        nc.vector.tensor_add(acc0[:], acc0[:], ej[:, 0:512])
                    if j in GP_H1:
                        if j == GP_H1[0]:
                            nc.gpsimd.tensor_copy(accg[:], ej[:, 512:1024])
                        else:
                            nc.gpsimd.tensor_add(accg[:], accg[:], ej[:, 512:1024])
                    else:
                        if j == 0:
                            nc.vector.tensor_copy(acc[:], ej[:, 512:1024])
                        else:
                            nc.vector.tensor_add(acc[:], acc[:], ej[:, 512:1024])
                if j >= LAG:
                    jj = j - LAG
                    ej = e_tiles.pop(jj)
                    for h in range(2):
                        nc.tensor.matmul(
                            out_ps[:, h * 512 : (h + 1) * 512],
                            vt_bf[:, 128 * jj : 128 * (jj + 1)],
                            ej[:, h * 512 : (h + 1) * 512],
                            start=(jj == 0),
                            stop=(jj == NCHUNK - 1),
                        )

            # cross-partition folds of the three accumulators
            nc.tensor.matmul(
                s_ps[0:1, :], ones_col[:], acc0[:], start=True, stop=True
            )
            sfold = stage.tile([128, 512], F32, tag="S")
            nc.tensor.matmul(
                sfold[0:1, :], ones_col[:], acc[:], start=True, stop=False
            )
            nc.tensor.matmul(
                sfold[0:1, :], ones_col[:], accg[:], start=False, stop=True
            )

            # tail in 512-wide halves: r = 1/s via the fast DVE reciprocal
            # (fp32, ~18 bits, reads s straight from PSUM), broadcast across
            # partitions with a K=1 PE matmul into a freed lpool bank and a
            # ScalarE copy back to SBUF (PE and ACT are both idle here, and
            # DVE may read only one PSUM operand), then a DVE multiply and
            # one fused multiply-add: out = (outPV * r) * gamma + zc' with
            # zc' = zc + gamma*bv folded on the host.  No exp/ln involved,
            # and gamma = 0 / gamma < 0 need no special casing.
            for h in range(2):
                sl = slice(h * 512, (h + 1) * 512)
                s_src = s_ps[0:1, :] if h == 0 else sfold[0:1, :]
                nc.vector.reciprocal_approx_fast(out=rvec[0:1, sl], in_=s_src)
                nc.gpsimd.partition_broadcast(rb_sb[:, sl], rvec[0:1, sl])
                nc.vector.tensor_tensor(
                    tmp_sb[:, sl], out_ps[:, sl], rb_sb[:, sl], op=ALU.mult
                )
                nc.vector.scalar_tensor_tensor(
                    out_sb[:, sl],
                    tmp_sb[:, sl],
                    gam_ap,
                    zc_sb[:, sl],
                    op0=ALU.mult,
                    op1=ALU.add,
                )
                # separate queues so the two output transfers overlap;
                # ScalarE is idle at the tail, GPSIMD is doing broadcasts
                eng = nc.sync if h == 0 else nc.scalar
                eng.dma_start(out_d[:, sl], out_sb[:, sl])

    nc.compile()
    return nc


_CACHE = {}


def _get_program(use_qk_bias: bool):
    if use_qk_bias not in _CACHE:
        _CACHE[use_qk_bias] = _build(use_qk_bias)
    return _CACHE[use_qk_bias]


def kernel(zc, zm, Wq, bq, Wk, bk, Wv, bv, gamma):
    global LAST_RESULTS
    zc = np.ascontiguousarray(zc, dtype=np.float32)
    zmf = np.asarray(zm, dtype=np.float32).reshape(B, CM, N)
    zmf_bf = zmf.astype(ml_dtypes.bfloat16)
    zcf = zc.reshape(B, CC, N)

    Wq = np.asarray(Wq, dtype=np.float32)
    Wk = np.asarray(Wk, dtype=np.float32)
    Wv = np.asarray(Wv, dtype=np.float32)
    # lps[n,m] = sum_c zm[c,n] tq[c,m] must equal k_n . q_m = zm_n^T (Wk^T Wq) zm_m,
    # so tq = (Wk^T Wq) zm_q; the tq matmul computes gt^T @ zm_q, hence
    # gt = (Wk^T Wq)^T = Wq^T Wk.
    gt = (Wq.astype(np.float64).T @ Wk.astype(np.float64)).astype(
        ml_dtypes.bfloat16
    )
    wvt = np.ascontiguousarray(Wv.T).astype(ml_dtypes.bfloat16)
    gamma_v = np.float32(np.asarray(gamma).reshape(-1)[0])
    sc_arr = np.full((CC, 1), gamma_v, dtype=np.float32)
    # zc' = zc + gamma*bv, so the kernel tail is a single multiply-add
    zcf = zcf + (gamma_v * np.asarray(bv, dtype=np.float32))[None, :, None]

    use_qk_bias = bool(np.any(bq)) or bool(np.any(bk))
    nc = _get_program(use_qk_bias)

    in_maps = []
    for c in range(NCORES):
        b, jblk = divmod(c, 4)
        m = {
            "zm": np.ascontiguousarray(
                np.roll(zmf_bf[b], -MBLK * jblk, axis=1)
            ),
            "zc": np.ascontiguousarray(zcf[b][:, MBLK * jblk : MBLK * (jblk + 1)]),
            "gt": gt,
            "wvt": wvt,
            "sc": sc_arr,
            "onesc": np.ones((128, 1), dtype=ml_dtypes.bfloat16),
        }
        if use_qk_bias:
            m["u"] = np.ascontiguousarray(
                (Wk.T @ np.asarray(bq, dtype=np.float32)).reshape(CM, 1)
            ).astype(ml_dtypes.bfloat16)
        in_maps.append(m)

    trace = bool(int(os.environ.get("BASS_KERNEL_TRACE", "0")))
    if trace and not _ensure_ntff_hook():
        trace = False
    res = run_bass_kernel_spmd(
        nc,
        in_maps,
        core_ids=list(range(NCORES)),
        trace=trace,
    )
    LAST_RESULTS = res

    out = np.empty((B, CC, N), dtype=np.float32)
    for c in range(NCORES):
        b, jblk = divmod(c, 4)
        out[b][:, MBLK * jblk : MBLK * (jblk + 1)] = res.results[c]["out"]
    return out.reshape(zc.shape)


# revision 65
# speedup vs baseline: 1.2408x; 1.2408x over previous
"""Trainium2 Bass kernel for nn_AttentionAggregator3d.

Math (per batch b):
    zmf = zm.reshape(CM, N)                     # N = D*W*H = 4096 tokens
    q = Wq @ zmf + bq ; k = Wk @ zmf + bk       # (16, N)
    v = Wv @ zmf + bv                           # (128, N)
    A = softmax_n(q^T k)                        # (N, N), softmax over keys n
    out = v @ A^T ; result = zc + gamma * out

Kernel structure (ScalarE-stream-bound design, ~58-60us/core):
  * logits^T[n, m] = k_n . q_m with G = Wk^T Wq folded on host; the
    query-side transform tq = G^T zm_q (128 x 1024) is computed ONCE and
    used as the bf16 MOVING operand of every logits matmul, with the key
    chunk zm_j as the stationary.  (The old version materialized t = G zm
    over all 4096 keys and burned 8 ScalarE copies staging it; ScalarE is
    the bottleneck engine, so those copies came straight out of the
    critical path.)
  * All matmul operands are bf16 (host pre-converts zm/G/Wv^T); exp output
    E is bf16 too, which halves SBUF traffic and lets the DVE denominator
    accumulation run in the 16-bit DVE perf modes.
  * Steady state: ScalarE streams 32 back-to-back [128,1024] exps
    (~1.05us each = the roofline); TensorE (~95% busy) does per chunk two
    512-wide logits matmuls, two 512-wide PV accumulations and an
    amortized 1/4 vproj batch; DVE accumulates both denominator halves
    except 8 early-chunk halves on GPSIMD.
  * Tail: three ones-matmul folds -> fast DVE reciprocal (straight from
    PSUM) -> GPSIMD partition_broadcast -> DVE multiply + fused
    multiply-add (gamma and gamma*bv are folded host-side into a single
    scale and into zc, so gamma=0 / negative gamma need no special path).
  * Sharding: 8 cores = batch (2) x query-block (4, 1024 queries each),
    zm rolled per core so its query block sits at columns 0:1024.
  * ACT tables are prefetched with a dummy exp at t=0 so the table load
    overlaps the input DMAs; the zm[:, 0:1024] DMA (which gates the first
    exp) is split across two queues because transfers serialize per queue.
"""

import os
import sys
import types

import ml_dtypes
import numpy as np

import concourse.bacc as bacc_mod
import concourse.tile as tile
from concourse import mybir
from concourse.bass_utils import run_bass_kernel_spmd

B, CC, CM, P = 2, 128, 128, 16
N = 16 * 16 * 16          # 4096 tokens
MBLK = N // 4             # 1024 queries per core
NCORES = 8
NCHUNK = N // 128         # 32 key chunks of 128

F32 = mybir.dt.float32
F32R = mybir.dt.float32r
BF16 = mybir.dt.bfloat16
AF = mybir.ActivationFunctionType
ALU = mybir.AluOpType

LAST_RESULTS = None  # BassKernelResults of the most recent run (for test.py)


def _ensure_ntff_hook() -> bool:
    """The grading image lacks antenv.axon_hooks; synthesize it from the
    boot module's ctypes NTFF driver so trace=True works under axon."""
    try:
        import antenv.axon_hooks  # noqa: F401

        return True
    except ImportError:
        pass
    try:
        import antenv
        from trn_agent_boot.trn_boot import _ntff_profile_via_ctypes

        hook = _ntff_profile_via_ctypes("/opt/axon/libaxon_pjrt.so")
        mod = types.ModuleType("antenv.axon_hooks")
        mod.get_axon_ntff_profile_hook = lambda: hook
        mod.set_axon_ntff_profile_hook = lambda h: None
        sys.modules["antenv.axon_hooks"] = mod
        antenv.axon_hooks = mod
        return hook is not None
    except Exception:
        return False


# Route Exp and Ln to the one table set that holds both, so the kernel pays a
# single ACT_TABLE_LOAD instead of three (exp -> ln -> exp again).
_orig_gat = bacc_mod.get_activation_tables
_COMBINED_SET = "natural_log_exp_and_others"


def _patched_gat(arch):
    tabs = _orig_gat(arch)
    if _COMBINED_SET in tabs:
        for name, fns in tabs.items():
            if name != _COMBINED_SET:
                fns.discard(AF.Exp)
                fns.discard(AF.Ln)
    return tabs


bacc_mod.get_activation_tables = _patched_gat


def _build(use_qk_bias: bool):
    nc = bacc_mod.Bacc(
        "TRN2",
        target_bir_lowering=False,
        debug=False,
        num_devices=NCORES,
    )

    zm_d = nc.dram_tensor("zm", (CM, N), BF16, kind="ExternalInput").ap()
    zc_d = nc.dram_tensor("zc", (CC, MBLK), F32, kind="ExternalInput").ap()
    gt_d = nc.dram_tensor("gt", (CM, CM), BF16, kind="ExternalInput").ap()
    wvt_d = nc.dram_tensor("wvt", (CM, CC), BF16, kind="ExternalInput").ap()
    # packed per-partition scalars: col 0 = gamma (gamma*bv is pre-added
    # into zc on the host)
    sc_d = nc.dram_tensor("sc", (CC, 1), F32, kind="ExternalInput").ap()
    onesc_d = nc.dram_tensor("onesc", (128, 1), BF16, kind="ExternalInput").ap()
    if use_qk_bias:
        u_d = nc.dram_tensor("u", (CM, 1), BF16, kind="ExternalInput").ap()
    out_d = nc.dram_tensor("out", (CC, MBLK), F32, kind="ExternalOutput").ap()

    # denominator routing (see module docstring): half 0 accumulates on the
    # DVE (acc0); half 1 goes to GPSIMD for early chunks (it is slow per
    # add, so keep it off the final-chunk critical path), else to the DVE
    # acc accumulator.  PE is the stream pacer at ~95%+ busy, so it gets no
    # ones-matmuls during the stream; the cross-partition folds happen in
    # three matmuls after the last exp.
    GP_H1 = [j for j in range(NCHUNK) if j % 3 == 1 and j < 24]

    with tile.TileContext(nc) as tc:
        with (
            tc.tile_pool(name="consts", bufs=1) as consts,
            tc.tile_pool(name="epool", bufs=10) as epool,
            tc.tile_pool(name="lpool", bufs=2, space="PSUM") as lpool,
            tc.tile_pool(name="stage", bufs=1, space="PSUM") as stage,
            tc.tile_pool(name="opool", bufs=1, space="PSUM") as opool,
            tc.tile_pool(name="spool", bufs=1, space="PSUM") as spool,
        ):
            zm_bf = consts.tile([CM, N], BF16, tag="zm")
            tq_bf = consts.tile([CM, MBLK], BF16, tag="tq")
            vt_bf = consts.tile([128, N], BF16, tag="vt")  # chunk j at cols 128j
            zc_sb = consts.tile([CC, MBLK], F32, tag="zc")
            gt_sb = consts.tile([CM, CM], BF16, tag="gt")
            wvt_sb = consts.tile([CM, CC], BF16, tag="wvt")
            sc_sb = consts.tile([CC, 1], F32, tag="sc")
            ones_col = consts.tile([128, 1], BF16, tag="onesc")
            acc0 = consts.tile([128, 512], BF16, tag="acc0")   # DVE, half 0
            acc = consts.tile([128, 512], BF16, tag="acc")     # DVE, half 1
            accg = consts.tile([128, 512], BF16, tag="accg")   # GPSIMD, half 1
            rvec = consts.tile([1, MBLK], F32, tag="rvec")
            rb_sb = consts.tile([128, MBLK], F32, tag="rb")
            tmp_sb = consts.tile([CC, MBLK], F32, tag="tmp")
            out_sb = consts.tile([CC, MBLK], F32, tag="outsb")
            scr = consts.tile([1, 1], F32, tag="scr")
            if use_qk_bias:
                u_sb = consts.tile([CM, 1], BF16, tag="u")
                rn_sb = consts.tile([128, NCHUNK], F32, tag="rn")

            # (A PE warm-up via junk matmuls during the DMA wait was tried
            # and removed: the HAM un-throttle lags the busy burst by ~2us,
            # so the dummies delay the first real matmul without making the
            # tq/logits-0 chain run warm.)

            # ---- input DMAs.  The critical chain is zm[:, 0:1024] + gt ->
            # tq -> cast -> logits 0 -> exp 0.  DMA *transfers* serialize per
            # issuing queue (~2.5us per 1024 bf16 columns), so the two tq
            # halves of zm go on different queues (sync / scalar) and the
            # rest follows on gpsimd, ordered by first use. ----
            nc.sync.dma_start(zm_bf[:, 0:512], zm_d[:, 0:512])
            nc.scalar.dma_start(gt_sb[:], gt_d)
            nc.scalar.dma_start(zm_bf[:, 512:1024], zm_d[:, 512:1024])

            # ACT table prefetch: a dummy exp with no real consumers makes
            # ScalarE pay the ~2.7us exp/ln table load during the input DMAs
            # instead of in front of the first real exp
            nc.scalar.activation(scr[0:1, 0:1], gt_sb[0:1, 0:1], AF.Exp)

            nc.sync.dma_start(ones_col[:], onesc_d)
            nc.gpsimd.dma_start(wvt_sb[:], wvt_d)
            if use_qk_bias:
                nc.gpsimd.dma_start(u_sb[:], u_d)
            nc.gpsimd.dma_start(zm_bf[:, 1024:2048], zm_d[:, 1024:2048])
            nc.gpsimd.dma_start(zm_bf[:, 2048:3072], zm_d[:, 2048:3072])
            nc.gpsimd.dma_start(zm_bf[:, 3072:4096], zm_d[:, 3072:4096])
            nc.gpsimd.dma_start(sc_sb[:], sc_d)
            nc.sync.dma_start(zc_sb[:], zc_d)

            gam_ap = sc_sb[:, 0:1]

            out_ps = opool.tile([CC, MBLK], F32, tag="out")
            s_ps = spool.tile([1, 512], F32, tag="s")  # half-0 denom fold

            # ---- tq = G zm_q over the core's 1024 query columns; cast in
            # halves so the DVE cast of half 0 overlaps the half-1 matmul ----
            tq_ps = lpool.tile([128, MBLK], F32, tag="L")
            for h in range(2):
                nc.tensor.matmul(
                    tq_ps[:, h * 512 : (h + 1) * 512],
                    gt_sb[:],
                    zm_bf[:, h * 512 : (h + 1) * 512],
                    start=True,
                    stop=True,
                )
                nc.vector.tensor_copy(
                    tq_bf[:, h * 512 : (h + 1) * 512],
                    tq_ps[:, h * 512 : (h + 1) * 512],
                )

            def emit_vt_batch(i):
                # vt chunk j = (zm chunk j)^T @ Wv^T for j in 4i..4i+3
                vps = stage.tile([128, 512], F32, tag="S")
                for k in range(4):
                    j = 4 * i + k
                    nc.tensor.matmul(
                        vps[:, 128 * k : 128 * (k + 1)],
                        zm_bf[:, 128 * j : 128 * (j + 1)],
                        wvt_sb[:],
                        start=True,
                        stop=True,
                    )
                nc.vector.tensor_copy(vt_bf[:, i * 512 : (i + 1) * 512], vps[:])
                if use_qk_bias:
                    rnps = stage.tile([128, 512], F32, tag="S")
                    for k in range(4):
                        j = 4 * i + k
                        nc.tensor.matmul(
                            rnps[:, k : k + 1],
                            zm_bf[:, 128 * j : 128 * (j + 1)],
                            u_sb[:],
                            start=True,
                            stop=True,
                        )
                    nc.vector.tensor_copy(
                        rn_sb[:, 4 * i : 4 * (i + 1)], rnps[:, 0:4]
                    )

            if use_qk_bias:
                # the chunk-0..3 exp biases must be materialized before the
                # first exp reads them
                emit_vt_batch(0)

            e_tiles = {}

            LAG = int(os.environ.get("BASS_PV_LAG", "2"))
            for j in range(NCHUNK + LAG):
                if j < NCHUNK:
                    if j % 4 == 2 and j // 4 + 1 <= 7:
                        emit_vt_batch(j // 4 + 1)
                    # logits^T chunk j: (keys 128, queries 1024)
                    lps = lpool.tile([128, MBLK], F32, tag="L")
                    for h in range(2):
                        nc.tensor.matmul(
                            lps[:, h * 512 : (h + 1) * 512],
                            zm_bf[:, 128 * j : 128 * (j + 1)],
                            tq_bf[:, h * 512 : (h + 1) * 512],
                            start=True,
                            stop=True,
                        )
                    ej = epool.tile([128, MBLK], BF16, tag="E")
                    bias = rn_sb[:, j : j + 1] if use_qk_bias else 0.0
                    nc.scalar.activation(ej[:], lps[:], AF.Exp, bias=bias)
                    e_tiles[j] = ej
                    if j == 1 and not use_qk_bias:
                        emit_vt_batch(0)
                    # softmax-denominator accumulation for chunk j
                    if j == 0:
                        nc.vector.tensor_copy(acc0[:], ej[:, 0:512])
                    else:
                        nc.vector.tensor_add(acc0[:], acc0[:], ej[:, 0:512])
                    if j in GP_H1:
                        if j == GP_H1[0]:
                            nc.gpsimd.tensor_copy(accg[:], ej[:, 512:1024])
                        else:
                            nc.gpsimd.tensor_add(accg[:], accg[:], ej[:, 512:1024])
                    else:
                        if j == 0:
                            nc.vector.tensor_copy(acc[:], ej[:, 512:1024])
                        else:
                            nc.vector.tensor_add(acc[:], acc[:], ej[:, 512:1024])
                if j >= LAG:
                    jj = j - LAG
                    ej = e_tiles.pop(jj)
                    for h in range(2):
                        nc.tensor.matmul(
                            out_ps[:, h * 512 : (h + 1) * 512],
                            vt_bf[:, 128 * jj : 128 * (jj + 1)],
                            ej[:, h * 512 : (h + 1) * 512],
                            start=(jj == 0),
                            stop=(jj == NCHUNK - 1),
                        )

            # cross-partition folds of the three accumulators
            nc.tensor.matmul(
                s_ps[0:1, :], ones_col[:], acc0[:], start=True, stop=True
            )
            sfold = stage.tile([128, 512], F32, tag="S")
            nc.tensor.matmul(
                sfold[0:1, :], ones_col[:], acc[:], start=True, stop=False
            )
            nc.tensor.matmul(
                sfold[0:1, :], ones_col[:], accg[:], start=False, stop=True
            )

            # tail in 512-wide halves: r = 1/s via the fast DVE reciprocal
            # (fp32, ~18 bits, reads s straight from PSUM), broadcast across
            # partitions with a K=1 PE matmul into a freed lpool bank and a
            # ScalarE copy back to SBUF (PE and ACT are both idle here, and
            # DVE may read only one PSUM operand), then a DVE multiply and
            # one fused multiply-add: out = (outPV * r) * gamma + zc' with
            # zc' = zc + gamma*bv folded on the host.  No exp/ln involved,
            # and gamma = 0 / gamma < 0 need no special casing.
            for h in range(2):
                sl = slice(h * 512, (h + 1) * 512)
                s_src = s_ps[0:1, :] if h == 0 else sfold[0:1, :]
                nc.vector.reciprocal_approx_fast(out=rvec[0:1, sl], in_=s_src)
                nc.gpsimd.partition_broadcast(rb_sb[:, sl], rvec[0:1, sl])
                nc.vector.tensor_tensor(
                    tmp_sb[:, sl], out_ps[:, sl], rb_sb[:, sl], op=ALU.mult
                )
                nc.vector.scalar_tensor_tensor(
                    out_sb[:, sl],
                    tmp_sb[:, sl],
                    gam_ap,
                    zc_sb[:, sl],
                    op0=ALU.mult,
                    op1=ALU.add,
                )
                # separate queues so the two output transfers overlap;
                # ScalarE is idle at the tail, GPSIMD is doing broadcasts
                eng = nc.sync if h == 0 else nc.scalar
                eng.dma_start(out_d[:, sl], out_sb[:, sl])

    nc.compile()
    return nc


_CACHE = {}


def _get_program(use_qk_bias: bool):
    if use_qk_bias not in _CACHE:
        _CACHE[use_qk_bias] = _build(use_qk_bias)
    return _CACHE[use_qk_bias]


def kernel(zc, zm, Wq, bq, Wk, bk, Wv, bv, gamma):
    global LAST_RESULTS
    zc = np.ascontiguousarray(zc, dtype=np.float32)
    zmf = np.asarray(zm, dtype=np.float32).reshape(B, CM, N)
    zmf_bf = zmf.astype(ml_dtypes.bfloat16)
    zcf = zc.reshape(B, CC, N)

    Wq = np.asarray(Wq, dtype=np.float32)
    Wk = np.asarray(Wk, dtype=np.float32)
    Wv = np.asarray(Wv, dtype=np.float32)
    # lps[n,m] = sum_c zm[c,n] tq[c,m] must equal k_n . q_m = zm_n^T (Wk^T Wq) zm_m,
    # so tq = (Wk^T Wq) zm_q; the tq matmul computes gt^T @ zm_q, hence
    # gt = (Wk^T Wq)^T = Wq^T Wk.
    gt = (Wq.astype(np.float64).T @ Wk.astype(np.float64)).astype(
        ml_dtypes.bfloat16
    )
    wvt = np.ascontiguousarray(Wv.T).astype(ml_dtypes.bfloat16)
    gamma_v = np.float32(np.asarray(gamma).reshape(-1)[0])
    sc_arr = np.full((CC, 1), gamma_v, dtype=np.float32)
    # zc' = zc + gamma*bv, so the kernel tail is a single multiply-add
    zcf = zcf + (gamma_v * np.asarray(bv, dtype=np.float32))[None, :, None]

    use_qk_bias = bool(np.any(bq)) or bool(np.any(bk))
    nc = _get_program(use_qk_bias)

    in_maps = []
    for c in range(NCORES):
        b, jblk = divmod(c, 4)
        m = {
            "zm": np.ascontiguousarray(
                np.roll(zmf_bf[b], -MBLK * jblk, axis=1)
            ),
            "zc": np.ascontiguousarray(zcf[b][:, MBLK * jblk : MBLK * (jblk + 1)]),
            "gt": gt,
            "wvt": wvt,
            "sc": sc_arr,
            "onesc": np.ones((128, 1), dtype=ml_dtypes.bfloat16),
        }
        if use_qk_bias:
            m["u"] = np.ascontiguousarray(
                (Wk.T @ np.asarray(bq, dtype=np.float32)).reshape(CM, 1)
            ).astype(ml_dtypes.bfloat16)
        in_maps.append(m)

    trace = bool(int(os.environ.get("BASS_KERNEL_TRACE", "0")))
    if trace and not _ensure_ntff_hook():
        trace = False
    res = run_bass_kernel_spmd(
        nc,
        in_maps,
        core_ids=list(range(NCORES)),
        trace=trace,
    )
    LAST_RESULTS = res

    out = np.empty((B, CC, N), dtype=np.float32)
    for c in range(NCORES):
        b, jblk = divmod(c, 4)
        out[b][:, MBLK * jblk : MBLK * (jblk + 1)] = res.results[c]["out"]
    return out.reshape(zc.shape)


# revision 69
# speedup vs baseline: 1.2417x; 1.0007x over previous
"""Trainium2 Bass kernel for nn_AttentionAggregator3d.

Math (per batch b):
    zmf = zm.reshape(CM, N)                     # N = D*W*H = 4096 tokens
    q = Wq @ zmf + bq ; k = Wk @ zmf + bk       # (16, N)
    v = Wv @ zmf + bv                           # (128, N)
    A = softmax_n(q^T k)                        # (N, N), softmax over keys n
    out = v @ A^T ; result = zc + gamma * out

Kernel structure (ScalarE-stream-bound design, ~58-60us/core):
  * logits^T[n, m] = k_n . q_m with G = Wk^T Wq folded on host; the
    query-side transform tq = G^T zm_q (128 x 1024) is computed ONCE and
    used as the bf16 MOVING operand of every logits matmul, with the key
    chunk zm_j as the stationary.  (The old version materialized t = G zm
    over all 4096 keys and burned 8 ScalarE copies staging it; ScalarE is
    the bottleneck engine, so those copies came straight out of the
    critical path.)
  * All matmul operands are bf16 (host pre-converts zm/G/Wv^T); exp output
    E is bf16 too, which halves SBUF traffic and lets the DVE denominator
    accumulation run in the 16-bit DVE perf modes.
  * Steady state: ScalarE streams 32 back-to-back [128,1024] exps
    (~1.05us each = the roofline); TensorE (~95% busy) does per chunk two
    512-wide logits matmuls, two 512-wide PV accumulations and an
    amortized 1/4 vproj batch; DVE accumulates both denominator halves
    except 8 early-chunk halves on GPSIMD.
  * Tail: three ones-matmul folds -> fast DVE reciprocal (straight from
    PSUM) -> GPSIMD partition_broadcast -> DVE multiply + fused
    multiply-add (gamma and gamma*bv are folded host-side into a single
    scale and into zc, so gamma=0 / negative gamma need no special path).
  * Sharding: 8 cores = batch (2) x query-block (4, 1024 queries each),
    zm rolled per core so its query block sits at columns 0:1024.
  * ACT tables are prefetched with a dummy exp at t=0 so the table load
    overlaps the input DMAs; the zm[:, 0:1024] DMA (which gates the first
    exp) is split across two queues because transfers serialize per queue.
"""

import os
import sys
import types

import ml_dtypes
import numpy as np

import concourse.bacc as bacc_mod
import concourse.tile as tile
from concourse import mybir
from concourse.bass_utils import run_bass_kernel_spmd

B, CC, CM, P = 2, 128, 128, 16
N = 16 * 16 * 16          # 4096 tokens
MBLK = N // 4             # 1024 queries per core
NCORES = 8
NCHUNK = N // 128         # 32 key chunks of 128

F32 = mybir.dt.float32
F32R = mybir.dt.float32r
BF16 = mybir.dt.bfloat16
AF = mybir.ActivationFunctionType
ALU = mybir.AluOpType

LAST_RESULTS = None  # BassKernelResults of the most recent run (for test.py)


def _ensure_ntff_hook() -> bool:
    """The grading image lacks antenv.axon_hooks; synthesize it from the
    boot module's ctypes NTFF driver so trace=True works under axon."""
    try:
        import antenv.axon_hooks  # noqa: F401

        return True
    except ImportError:
        pass
    try:
        import antenv
        from trn_agent_boot.trn_boot import _ntff_profile_via_ctypes

        hook = _ntff_profile_via_ctypes("/opt/axon/libaxon_pjrt.so")
        mod = types.ModuleType("antenv.axon_hooks")
        mod.get_axon_ntff_profile_hook = lambda: hook
        mod.set_axon_ntff_profile_hook = lambda h: None
        sys.modules["antenv.axon_hooks"] = mod
        antenv.axon_hooks = mod
        return hook is not None
    except Exception:
        return False


# Route Exp and Ln to the one table set that holds both, so the kernel pays a
# single ACT_TABLE_LOAD instead of three (exp -> ln -> exp again).
_orig_gat = bacc_mod.get_activation_tables
_COMBINED_SET = "natural_log_exp_and_others"


def _patched_gat(arch):
    tabs = _orig_gat(arch)
    if _COMBINED_SET in tabs:
        for name, fns in tabs.items():
            if name != _COMBINED_SET:
                fns.discard(AF.Exp)
                fns.discard(AF.Ln)
    return tabs


bacc_mod.get_activation_tables = _patched_gat


def _build(use_qk_bias: bool):
    nc = bacc_mod.Bacc(
        "TRN2",
        target_bir_lowering=False,
        debug=False,
        num_devices=NCORES,
    )

    zm_d = nc.dram_tensor("zm", (CM, N), BF16, kind="ExternalInput").ap()
    zc_d = nc.dram_tensor("zc", (CC, MBLK), F32, kind="ExternalInput").ap()
    gt_d = nc.dram_tensor("gt", (CM, CM), BF16, kind="ExternalInput").ap()
    wvt_d = nc.dram_tensor("wvt", (CM, CC), BF16, kind="ExternalInput").ap()
    # packed per-partition scalars: col 0 = gamma (gamma*bv is pre-added
    # into zc on the host)
    sc_d = nc.dram_tensor("sc", (CC, 1), F32, kind="ExternalInput").ap()
    onesc_d = nc.dram_tensor("onesc", (128, 1), BF16, kind="ExternalInput").ap()
    if use_qk_bias:
        u_d = nc.dram_tensor("u", (CM, 1), BF16, kind="ExternalInput").ap()
    out_d = nc.dram_tensor("out", (CC, MBLK), F32, kind="ExternalOutput").ap()

    # denominator routing (see module docstring): half 0 accumulates on the
    # DVE (acc0); half 1 goes to GPSIMD for early chunks (it is slow per
    # add, so keep it off the final-chunk critical path), else to the DVE
    # acc accumulator.  PE is the stream pacer at ~95%+ busy, so it gets no
    # ones-matmuls during the stream; the cross-partition folds happen in
    # three matmuls after the last exp.
    GP_H1 = [j for j in range(NCHUNK) if j % 3 == 1 and j < 24]

    with tile.TileContext(nc) as tc:
        with (
            tc.tile_pool(name="consts", bufs=1) as consts,
            tc.tile_pool(name="epool", bufs=6) as epool,
            tc.tile_pool(name="lpa", bufs=1, space="PSUM") as lpa,
            tc.tile_pool(name="lpb", bufs=1, space="PSUM") as lpb,
            tc.tile_pool(name="stage", bufs=1, space="PSUM") as stage,
            tc.tile_pool(name="opool", bufs=1, space="PSUM") as opool,
        ):
            zm_bf = consts.tile([CM, N], BF16, tag="zm")
            tq_bf = consts.tile([CM, MBLK], BF16, tag="tq")
            vt_bf = consts.tile([128, N], BF16, tag="vt")  # chunk j at cols 128j
            zc_sb = consts.tile([CC, MBLK], F32, tag="zc")
            gt_sb = consts.tile([CM, CM], BF16, tag="gt")
            wvt_sb = consts.tile([CM, CC], BF16, tag="wvt")
            sc_sb = consts.tile([CC, 1], F32, tag="sc")
            ones_col = consts.tile([128, 1], BF16, tag="onesc")
            acc0 = consts.tile([128, 512], BF16, tag="acc0")   # DVE, half 0
            acc = consts.tile([128, 512], BF16, tag="acc")     # DVE, half 1
            accg = consts.tile([128, 512], BF16, tag="accg")   # GPSIMD, half 1
            rvec = consts.tile([1, MBLK], F32, tag="rvec")
            rb_sb = consts.tile([128, MBLK], F32, tag="rb")
            tmp_sb = consts.tile([CC, MBLK], F32, tag="tmp")
            out_sb = consts.tile([CC, MBLK], F32, tag="outsb")
            scr = consts.tile([1, 1], F32, tag="scr")
            if use_qk_bias:
                u_sb = consts.tile([CM, 1], BF16, tag="u")
                rn_sb = consts.tile([128, NCHUNK], F32, tag="rn")

            # (A PE warm-up via junk matmuls during the DMA wait was tried
            # and removed: the HAM un-throttle lags the busy burst by ~2us,
            # so the dummies delay the first real matmul without making the
            # tq/logits-0 chain run warm.)

            # ---- input DMAs.  The critical chain is zm[:, 0:1024] + gt ->
            # tq -> cast -> logits 0 -> exp 0.  DMA *transfers* serialize per
            # issuing queue (~2.5us per 1024 bf16 columns), so the two tq
            # halves of zm go on different queues (sync / scalar) and the
            # rest follows on gpsimd, ordered by first use. ----
            nc.sync.dma_start(zm_bf[:, 0:512], zm_d[:, 0:512])
            nc.scalar.dma_start(gt_sb[:], gt_d)
            nc.scalar.dma_start(zm_bf[:, 512:1024], zm_d[:, 512:1024])

            # ACT table prefetch: a dummy exp with no real consumers makes
            # ScalarE pay the ~2.7us exp/ln table load during the input DMAs
            # instead of in front of the first real exp
            nc.scalar.activation(scr[0:1, 0:1], gt_sb[0:1, 0:1], AF.Exp)

            nc.sync.dma_start(ones_col[:], onesc_d)
            nc.gpsimd.dma_start(wvt_sb[:], wvt_d)
            if use_qk_bias:
                nc.gpsimd.dma_start(u_sb[:], u_d)
            nc.gpsimd.dma_start(zm_bf[:, 1024:2048], zm_d[:, 1024:2048])
            nc.gpsimd.dma_start(zm_bf[:, 2048:3072], zm_d[:, 2048:3072])
            nc.gpsimd.dma_start(zm_bf[:, 3072:4096], zm_d[:, 3072:4096])
            nc.gpsimd.dma_start(sc_sb[:], sc_d)
            nc.sync.dma_start(zc_sb[:], zc_d)

            gam_ap = sc_sb[:, 0:1]

            out_ps = opool.tile([CC, MBLK], F32, tag="out")

            # ---- tq = G zm_q over the core's 1024 query columns; cast in
            # halves so the DVE cast of half 0 overlaps the half-1 matmul ----
            tq_ps = lpa.tile([128, 1536], F32, tag="P")
            for h in range(2):
                nc.tensor.matmul(
                    tq_ps[:, h * 512 : (h + 1) * 512],
                    gt_sb[:],
                    zm_bf[:, h * 512 : (h + 1) * 512],
                    start=True,
                    stop=True,
                )
                nc.vector.tensor_copy(
                    tq_bf[:, h * 512 : (h + 1) * 512],
                    tq_ps[:, h * 512 : (h + 1) * 512],
                )

            def emit_vt_batch(i):
                # vt chunk j = (zm chunk j)^T @ Wv^T for j in 4i..4i+3
                vps = stage.tile([128, 512], F32, tag="S")
                for k in range(4):
                    j = 4 * i + k
                    nc.tensor.matmul(
                        vps[:, 128 * k : 128 * (k + 1)],
                        zm_bf[:, 128 * j : 128 * (j + 1)],
                        wvt_sb[:],
                        start=True,
                        stop=True,
                    )
                nc.vector.tensor_copy(vt_bf[:, i * 512 : (i + 1) * 512], vps[:])
                if use_qk_bias:
                    rnps = stage.tile([128, 512], F32, tag="S")
                    for k in range(4):
                        j = 4 * i + k
                        nc.tensor.matmul(
                            rnps[:, k : k + 1],
                            zm_bf[:, 128 * j : 128 * (j + 1)],
                            u_sb[:],
                            start=True,
                            stop=True,
                        )
                    nc.vector.tensor_copy(
                        rn_sb[:, 4 * i : 4 * (i + 1)], rnps[:, 0:4]
                    )

            if use_qk_bias:
                # the chunk-0..3 exp biases must be materialized before the
                # first exp reads them
                emit_vt_batch(0)

            # The exp stream is organized in UNITS of 512-wide half-chunk
            # slices (slice s = (chunk s//2, query-half s%2)).  Wider
            # ACTIVATEs amortize the ~185ns per-instruction init, but PSUM
            # allows only one 3-bank tile next to the 2-bank one (lpa+lpb+
            # stage+opool = 8 banks), so units alternate 1024/1536 wide.
            # Both unit pools are single-buffered; overlap comes from
            # alternating between them, and PV runs at a lag of TWO units so
            # the next unit's logits matmuls always precede PV work on the
            # PE queue (otherwise the 3-matmul refill of the wide tile
            # cannot hide inside the narrow exp's window).
            # The qk-bias variant needs a per-chunk exp bias, so it uses
            # chunk-aligned 1024-wide units only.
            if use_qk_bias:
                sizes = [2] * 32
            else:
                sizes = [2, 3] * 12 + [2, 2]
            units = []
            s0 = 0
            for sz in sizes:
                units.append(list(range(s0, s0 + sz)))
                s0 += sz
            assert s0 == 2 * NCHUNK

            e_hist = {}
            for k in range(len(units) + 2):
                if k < len(units):
                    sls = units[k]
                    w = 512 * len(sls)
                    if use_qk_bias:
                        big = k % 2 == 1
                    else:
                        big = len(sls) == 3
                    if big:
                        lps = lpa.tile([128, 1536], F32, tag="P")
                    else:
                        lps = lpb.tile([128, MBLK], F32, tag="Lh")
                    for i, sl in enumerate(sls):
                        c, hh = divmod(sl, 2)
                        nc.tensor.matmul(
                            lps[:, 512 * i : 512 * (i + 1)],
                            zm_bf[:, 128 * c : 128 * (c + 1)],
                            tq_bf[:, 512 * hh : 512 * (hh + 1)],
                            start=True,
                            stop=True,
                        )
                    ej = epool.tile([128, w], BF16, tag=f"E{len(sls)}")
                    bias = (
                        rn_sb[:, sls[0] // 2 : sls[0] // 2 + 1]
                        if use_qk_bias
                        else 0.0
                    )
                    nc.scalar.activation(ej[:], lps[:, 0:w], AF.Exp, bias=bias)
                    e_hist[k] = (ej, sls)
                    # value-projection batches, triggered by chunk progress
                    for sl in sls:
                        c, hh = divmod(sl, 2)
                        if hh != 0:
                            continue
                        if c == 1 and not use_qk_bias:
                            emit_vt_batch(0)
                        elif c % 4 == 2 and c // 4 + 1 <= 7:
                            emit_vt_batch(c // 4 + 1)
                    # softmax-denominator accumulation
                    for i, sl in enumerate(sls):
                        col = slice(512 * i, 512 * (i + 1))
                        c, hh = divmod(sl, 2)
                        if hh == 0:
                            if sl == 0:
                                nc.vector.tensor_copy(acc0[:], ej[:, col])
                            else:
                                nc.vector.tensor_add(acc0[:], acc0[:], ej[:, col])
                        elif c in GP_H1:
                            if c == GP_H1[0]:
                                nc.gpsimd.tensor_copy(accg[:], ej[:, col])
                            else:
                                nc.gpsimd.tensor_add(accg[:], accg[:], ej[:, col])
                        else:
                            if sl == 1:
                                nc.vector.tensor_copy(acc[:], ej[:, col])
                            else:
                                nc.vector.tensor_add(acc[:], acc[:], ej[:, col])
                if k >= 2:
                    ejp, slsp = e_hist.pop(k - 2)
                    for i, sl in enumerate(slsp):
                        c, hh = divmod(sl, 2)
                        nc.tensor.matmul(
                            out_ps[:, 512 * hh : 512 * (hh + 1)],
                            vt_bf[:, 128 * c : 128 * (c + 1)],
                            ejp[:, 512 * i : 512 * (i + 1)],
                            start=(sl <= 1),
                            stop=(sl >= 2 * NCHUNK - 2),
                        )

            # cross-partition folds of the three accumulators (into stream
            # banks freed by the finished exp units)
            s_ps = stage.tile([128, 512], F32, tag="S")
            nc.tensor.matmul(
                s_ps[0:1, 0:512], ones_col[:], acc0[:], start=True, stop=True
            )
            sfold = lpb.tile([128, MBLK], F32, tag="Lh")
            nc.tensor.matmul(
                sfold[0:1, 0:512], ones_col[:], acc[:], start=True, stop=False
            )
            nc.tensor.matmul(
                sfold[0:1, 0:512], ones_col[:], accg[:], start=False, stop=True
            )

            # tail in 512-wide halves: r = 1/s via the fast DVE reciprocal
            # (fp32, ~18 bits, reads s straight from PSUM), broadcast across
            # partitions with a K=1 PE matmul into a freed lpool bank and a
            # ScalarE copy back to SBUF (PE and ACT are both idle here, and
            # DVE may read only one PSUM operand), then a DVE multiply and
            # one fused multiply-add: out = (outPV * r) * gamma + zc' with
            # zc' = zc + gamma*bv folded on the host.  No exp/ln involved,
            # and gamma = 0 / gamma < 0 need no special casing.
            for h in range(2):
                sl = slice(h * 512, (h + 1) * 512)
                s_src = s_ps[0:1, 0:512] if h == 0 else sfold[0:1, 0:512]
                nc.vector.reciprocal_approx_fast(out=rvec[0:1, sl], in_=s_src)
                nc.gpsimd.partition_broadcast(rb_sb[:, sl], rvec[0:1, sl])
                nc.vector.tensor_tensor(
                    tmp_sb[:, sl], out_ps[:, sl], rb_sb[:, sl], op=ALU.mult
                )
                nc.vector.scalar_tensor_tensor(
                    out_sb[:, sl],
                    tmp_sb[:, sl],
                    gam_ap,
                    zc_sb[:, sl],
                    op0=ALU.mult,
                    op1=ALU.add,
                )
                # separate queues so the two output transfers overlap;
                # ScalarE is idle at the tail, GPSIMD is doing broadcasts
                eng = nc.sync if h == 0 else nc.scalar
                eng.dma_start(out_d[:, sl], out_sb[:, sl])

    nc.compile()
    return nc


_CACHE = {}


def _get_program(use_qk_bias: bool):
    if use_qk_bias not in _CACHE:
        _CACHE[use_qk_bias] = _build(use_qk_bias)
    return _CACHE[use_qk_bias]


def kernel(zc, zm, Wq, bq, Wk, bk, Wv, bv, gamma):
    global LAST_RESULTS
    zc = np.ascontiguousarray(zc, dtype=np.float32)
    zmf = np.asarray(zm, dtype=np.float32).reshape(B, CM, N)
    zmf_bf = zmf.astype(ml_dtypes.bfloat16)
    zcf = zc.reshape(B, CC, N)

    Wq = np.asarray(Wq, dtype=np.float32)
    Wk = np.asarray(Wk, dtype=np.float32)
    Wv = np.asarray(Wv, dtype=np.float32)
    # lps[n,m] = sum_c zm[c,n] tq[c,m] must equal k_n . q_m = zm_n^T (Wk^T Wq) zm_m,
    # so tq = (Wk^T Wq) zm_q; the tq matmul computes gt^T @ zm_q, hence
    # gt = (Wk^T Wq)^T = Wq^T Wk.
    gt = (Wq.astype(np.float64).T @ Wk.astype(np.float64)).astype(
        ml_dtypes.bfloat16
    )
    wvt = np.ascontiguousarray(Wv.T).astype(ml_dtypes.bfloat16)
    gamma_v = np.float32(np.asarray(gamma).reshape(-1)[0])
    sc_arr = np.full((CC, 1), gamma_v, dtype=np.float32)
    # zc' = zc + gamma*bv, so the kernel tail is a single multiply-add
    zcf = zcf + (gamma_v * np.asarray(bv, dtype=np.float32))[None, :, None]

    use_qk_bias = bool(np.any(bq)) or bool(np.any(bk))
    nc = _get_program(use_qk_bias)

    in_maps = []
    for c in range(NCORES):
        b, jblk = divmod(c, 4)
        m = {
            "zm": np.ascontiguousarray(
                np.roll(zmf_bf[b], -MBLK * jblk, axis=1)
            ),
            "zc": np.ascontiguousarray(zcf[b][:, MBLK * jblk : MBLK * (jblk + 1)]),
            "gt": gt,
            "wvt": wvt,
            "sc": sc_arr,
            "onesc": np.ones((128, 1), dtype=ml_dtypes.bfloat16),
        }
        if use_qk_bias:
            m["u"] = np.ascontiguousarray(
                (Wk.T @ np.asarray(bq, dtype=np.float32)).reshape(CM, 1)
            ).astype(ml_dtypes.bfloat16)
        in_maps.append(m)

    trace = bool(int(os.environ.get("BASS_KERNEL_TRACE", "0")))
    if trace and not _ensure_ntff_hook():
        trace = False
    res = run_bass_kernel_spmd(
        nc,
        in_maps,
        core_ids=list(range(NCORES)),
        trace=trace,
    )
    LAST_RESULTS = res

    out = np.empty((B, CC, N), dtype=np.float32)
    for c in range(NCORES):
        b, jblk = divmod(c, 4)
        out[b][:, MBLK * jblk : MBLK * (jblk + 1)] = res.results[c]["out"]
    return out.reshape(zc.shape)


# revision 73
# speedup vs baseline: 1.2595x; 1.0144x over previous
"""Trainium2 Bass kernel for nn_AttentionAggregator3d.

Math (per batch b):
    zmf = zm.reshape(CM, N)                     # N = D*W*H = 4096 tokens
    q = Wq @ zmf + bq ; k = Wk @ zmf + bk       # (16, N)
    v = Wv @ zmf + bv                           # (128, N)
    A = softmax_n(q^T k)                        # (N, N), softmax over keys n
    out = v @ A^T ; result = zc + gamma * out

Kernel structure (ScalarE-stream-bound design, ~58-60us/core):
  * logits^T[n, m] = k_n . q_m with G = Wk^T Wq folded on host; the
    query-side transform tq = G^T zm_q (128 x 1024) is computed ONCE and
    used as the bf16 MOVING operand of every logits matmul, with the key
    chunk zm_j as the stationary.  (The old version materialized t = G zm
    over all 4096 keys and burned 8 ScalarE copies staging it; ScalarE is
    the bottleneck engine, so those copies came straight out of the
    critical path.)
  * All matmul operands are bf16 (host pre-converts zm/G/Wv^T); exp output
    E is bf16 too, which halves SBUF traffic and lets the DVE denominator
    accumulation run in the 16-bit DVE perf modes.
  * Steady state: ScalarE streams 32 back-to-back [128,1024] exps
    (~1.05us each = the roofline); TensorE (~95% busy) does per chunk two
    512-wide logits matmuls, two 512-wide PV accumulations and an
    amortized 1/4 vproj batch; DVE accumulates both denominator halves
    except 8 early-chunk halves on GPSIMD.
  * Tail: three ones-matmul folds -> fast DVE reciprocal (straight from
    PSUM) -> GPSIMD partition_broadcast -> DVE multiply + fused
    multiply-add (gamma and gamma*bv are folded host-side into a single
    scale and into zc, so gamma=0 / negative gamma need no special path).
  * Sharding: 8 cores = batch (2) x query-block (4, 1024 queries each),
    zm rolled per core so its query block sits at columns 0:1024.
  * ACT tables are prefetched with a dummy exp at t=0 so the table load
    overlaps the input DMAs; the zm[:, 0:1024] DMA (which gates the first
    exp) is split across two queues because transfers serialize per queue.
"""

import os
import sys
import types

import ml_dtypes
import numpy as np

import concourse.bacc as bacc_mod
import concourse.tile as tile
from concourse import mybir
from concourse.bass_utils import run_bass_kernel_spmd

B, CC, CM, P = 2, 128, 128, 16
N = 16 * 16 * 16          # 4096 tokens
MBLK = N // 4             # 1024 queries per core
NCORES = 8
NCHUNK = N // 128         # 32 key chunks of 128

F32 = mybir.dt.float32
F32R = mybir.dt.float32r
BF16 = mybir.dt.bfloat16
AF = mybir.ActivationFunctionType
ALU = mybir.AluOpType

LAST_RESULTS = None  # BassKernelResults of the most recent run (for test.py)


def _ensure_ntff_hook() -> bool:
    """The grading image lacks antenv.axon_hooks; synthesize it from the
    boot module's ctypes NTFF driver so trace=True works under axon."""
    try:
        import antenv.axon_hooks  # noqa: F401

        return True
    except ImportError:
        pass
    try:
        import antenv
        from trn_agent_boot.trn_boot import _ntff_profile_via_ctypes

        hook = _ntff_profile_via_ctypes("/opt/axon/libaxon_pjrt.so")
        mod = types.ModuleType("antenv.axon_hooks")
        mod.get_axon_ntff_profile_hook = lambda: hook
        mod.set_axon_ntff_profile_hook = lambda h: None
        sys.modules["antenv.axon_hooks"] = mod
        antenv.axon_hooks = mod
        return hook is not None
    except Exception:
        return False


# Route Exp and Ln to the one table set that holds both, so the kernel pays a
# single ACT_TABLE_LOAD instead of three (exp -> ln -> exp again).
_orig_gat = bacc_mod.get_activation_tables
_COMBINED_SET = "natural_log_exp_and_others"


def _patched_gat(arch):
    tabs = _orig_gat(arch)
    if _COMBINED_SET in tabs:
        for name, fns in tabs.items():
            if name != _COMBINED_SET:
                fns.discard(AF.Exp)
                fns.discard(AF.Ln)
    return tabs


bacc_mod.get_activation_tables = _patched_gat


def _build(use_qk_bias: bool):
    nc = bacc_mod.Bacc(
        "TRN2",
        target_bir_lowering=False,
        debug=False,
        num_devices=NCORES,
    )

    zm_d = nc.dram_tensor("zm", (CM, N), BF16, kind="ExternalInput").ap()
    zc_d = nc.dram_tensor("zc", (CC, MBLK), F32, kind="ExternalInput").ap()
    gt_d = nc.dram_tensor("gt", (CM, CM), BF16, kind="ExternalInput").ap()
    wvt_d = nc.dram_tensor("wvt", (CM, CC), BF16, kind="ExternalInput").ap()
    # packed per-partition scalars: col 0 = gamma (gamma*bv is pre-added
    # into zc on the host)
    sc_d = nc.dram_tensor("sc", (CC, 1), F32, kind="ExternalInput").ap()
    onesc_d = nc.dram_tensor("onesc", (128, 1), BF16, kind="ExternalInput").ap()
    if use_qk_bias:
        u_d = nc.dram_tensor("u", (CM, 1), BF16, kind="ExternalInput").ap()
    out_d = nc.dram_tensor("out", (CC, MBLK), F32, kind="ExternalOutput").ap()

    # denominator routing (see module docstring): half 0 accumulates on the
    # DVE (acc0); half 1 goes to GPSIMD for early chunks (it is slow per
    # add, so keep it off the final-chunk critical path), else to the DVE
    # acc accumulator.  PE is the stream pacer at ~95%+ busy, so it gets no
    # ones-matmuls during the stream; the cross-partition folds happen in
    # three matmuls after the last exp.
    GP_H1 = [j for j in range(NCHUNK) if j % 3 == 1 and j < 24]

    with tile.TileContext(nc) as tc:
        with (
            tc.tile_pool(name="consts", bufs=1) as consts,
            tc.tile_pool(name="epool", bufs=6) as epool,
            tc.tile_pool(name="lpa", bufs=1, space="PSUM") as lpa,
            tc.tile_pool(name="lpb", bufs=1, space="PSUM") as lpb,
            tc.tile_pool(name="stage", bufs=1, space="PSUM") as stage,
            tc.tile_pool(name="opool", bufs=1, space="PSUM") as opool,
        ):
            zm_bf = consts.tile([CM, N], BF16, tag="zm")
            tq_bf = consts.tile([CM, MBLK], BF16, tag="tq")
            vt_bf = consts.tile([128, N], BF16, tag="vt")  # chunk j at cols 128j
            zc_sb = consts.tile([CC, MBLK], F32, tag="zc")
            gt_sb = consts.tile([CM, CM], BF16, tag="gt")
            wvt_sb = consts.tile([CM, CC], BF16, tag="wvt")
            sc_sb = consts.tile([CC, 1], F32, tag="sc")
            ones_col = consts.tile([128, 1], BF16, tag="onesc")
            acc0 = consts.tile([128, 512], BF16, tag="acc0")   # DVE, half 0
            acc = consts.tile([128, 512], BF16, tag="acc")     # DVE, half 1
            accg = consts.tile([128, 512], BF16, tag="accg")   # GPSIMD, half 1
            rvec = consts.tile([1, MBLK], F32, tag="rvec")
            rb_sb = consts.tile([128, MBLK], F32, tag="rb")
            tmp_sb = consts.tile([CC, MBLK], F32, tag="tmp")
            out_sb = consts.tile([CC, MBLK], F32, tag="outsb")
            scr = consts.tile([1, 1], F32, tag="scr")
            if use_qk_bias:
                u_sb = consts.tile([CM, 1], BF16, tag="u")
                rn_sb = consts.tile([128, NCHUNK], F32, tag="rn")

            # (A PE warm-up via junk matmuls during the DMA wait was tried
            # and removed: the HAM un-throttle lags the busy burst by ~2us,
            # so the dummies delay the first real matmul without making the
            # tq/logits-0 chain run warm.)

            # ---- input DMAs.  The critical chain is zm[:, 0:1024] + gt ->
            # tq -> cast -> logits 0 -> exp 0.  DMA *transfers* serialize per
            # issuing queue (~2.5us per 1024 bf16 columns), so the two tq
            # halves of zm go on different queues (sync / scalar) and the
            # rest follows on gpsimd, ordered by first use. ----
            nc.sync.dma_start(zm_bf[:, 0:512], zm_d[:, 0:512])
            nc.scalar.dma_start(gt_sb[:], gt_d)
            nc.scalar.dma_start(zm_bf[:, 512:1024], zm_d[:, 512:1024])

            # ACT table prefetch: a dummy exp with no real consumers makes
            # ScalarE pay the ~2.7us exp/ln table load during the input DMAs
            # instead of in front of the first real exp
            nc.scalar.activation(scr[0:1, 0:1], gt_sb[0:1, 0:1], AF.Exp)

            nc.sync.dma_start(ones_col[:], onesc_d)
            nc.gpsimd.dma_start(wvt_sb[:], wvt_d)
            if use_qk_bias:
                nc.gpsimd.dma_start(u_sb[:], u_d)
            nc.gpsimd.dma_start(zm_bf[:, 1024:2048], zm_d[:, 1024:2048])
            nc.gpsimd.dma_start(zm_bf[:, 2048:3072], zm_d[:, 2048:3072])
            nc.gpsimd.dma_start(zm_bf[:, 3072:4096], zm_d[:, 3072:4096])
            nc.gpsimd.dma_start(sc_sb[:], sc_d)
            nc.sync.dma_start(zc_sb[:], zc_d)

            gam_ap = sc_sb[:, 0:1]

            out_ps = opool.tile([CC, MBLK], F32, tag="out")

            # ---- tq = G zm_q over the core's 1024 query columns; cast in
            # halves so the DVE cast of half 0 overlaps the half-1 matmul ----
            tq_ps = lpa.tile([128, 1536], F32, tag="P")
            for h in range(2):
                nc.tensor.matmul(
                    tq_ps[:, h * 512 : (h + 1) * 512],
                    gt_sb[:],
                    zm_bf[:, h * 512 : (h + 1) * 512],
                    start=True,
                    stop=True,
                )
                nc.vector.tensor_copy(
                    tq_bf[:, h * 512 : (h + 1) * 512],
                    tq_ps[:, h * 512 : (h + 1) * 512],
                )

            def emit_vt_batch(i):
                # vt chunk j = (zm chunk j)^T @ Wv^T for j in 4i..4i+3
                vps = stage.tile([128, 512], F32, tag="S")
                for k in range(4):
                    j = 4 * i + k
                    nc.tensor.matmul(
                        vps[:, 128 * k : 128 * (k + 1)],
                        zm_bf[:, 128 * j : 128 * (j + 1)],
                        wvt_sb[:],
                        start=True,
                        stop=True,
                    )
                nc.vector.tensor_copy(vt_bf[:, i * 512 : (i + 1) * 512], vps[:])
                if use_qk_bias:
                    rnps = stage.tile([128, 512], F32, tag="S")
                    for k in range(4):
                        j = 4 * i + k
                        nc.tensor.matmul(
                            rnps[:, k : k + 1],
                            zm_bf[:, 128 * j : 128 * (j + 1)],
                            u_sb[:],
                            start=True,
                            stop=True,
                        )
                    nc.vector.tensor_copy(
                        rn_sb[:, 4 * i : 4 * (i + 1)], rnps[:, 0:4]
                    )

            if use_qk_bias:
                # the chunk-0..3 exp biases must be materialized before the
                # first exp reads them
                emit_vt_batch(0)

            # The exp stream is organized in UNITS of 512-wide half-chunk
            # slices (slice s = (chunk s//2, query-half s%2)).  Wider
            # ACTIVATEs amortize the ~185ns per-instruction init, but PSUM
            # allows only one 3-bank tile next to the 2-bank one (lpa+lpb+
            # stage+opool = 8 banks), so units alternate 1024/1536 wide.
            # Both unit pools are single-buffered; overlap comes from
            # alternating between them, and PV runs at a lag of TWO units so
            # the next unit's logits matmuls always precede PV work on the
            # PE queue (otherwise the 3-matmul refill of the wide tile
            # cannot hide inside the narrow exp's window).
            # The qk-bias variant needs a per-chunk exp bias, so it uses
            # chunk-aligned 1024-wide units only.
            # (size, pool) per unit; pools must strictly alternate or two
            # consecutive units share a single-buffered tile and serialize
            # the exp stream.  The odd slice count is absorbed by hosting
            # one narrow unit in the wide pool right after tq.
            if use_qk_bias:
                spec = [(2, "b"), (2, "a")] * 16
            else:
                spec = [(2, "b"), (2, "a")] + [(2, "b"), (3, "a")] * 12
            units = []
            s0 = 0
            for sz, pl in spec:
                units.append((list(range(s0, s0 + sz)), pl))
                s0 += sz
            assert s0 == 2 * NCHUNK

            e_hist = {}
            for k in range(len(units) + 2):
                if k < len(units):
                    sls, pl = units[k]
                    w = 512 * len(sls)
                    if pl == "a":
                        lps = lpa.tile([128, 1536], F32, tag="P")
                    else:
                        lps = lpb.tile([128, MBLK], F32, tag="Lh")
                    for i, sl in enumerate(sls):
                        c, hh = divmod(sl, 2)
                        nc.tensor.matmul(
                            lps[:, 512 * i : 512 * (i + 1)],
                            zm_bf[:, 128 * c : 128 * (c + 1)],
                            tq_bf[:, 512 * hh : 512 * (hh + 1)],
                            start=True,
                            stop=True,
                        )
                    ej = epool.tile([128, w], BF16, tag=f"E{len(sls)}")
                    bias = (
                        rn_sb[:, sls[0] // 2 : sls[0] // 2 + 1]
                        if use_qk_bias
                        else 0.0
                    )
                    nc.scalar.activation(ej[:], lps[:, 0:w], AF.Exp, bias=bias)
                    e_hist[k] = (ej, sls)
                    # value-projection batches, triggered by chunk progress
                    for sl in sls:
                        c, hh = divmod(sl, 2)
                        if hh != 0:
                            continue
                        if c == 1 and not use_qk_bias:
                            emit_vt_batch(0)
                        elif c % 4 == 2 and c // 4 + 1 <= 7:
                            emit_vt_batch(c // 4 + 1)
                    # softmax-denominator accumulation
                    for i, sl in enumerate(sls):
                        col = slice(512 * i, 512 * (i + 1))
                        c, hh = divmod(sl, 2)
                        if hh == 0:
                            if sl == 0:
                                nc.vector.tensor_copy(acc0[:], ej[:, col])
                            else:
                                nc.vector.tensor_add(acc0[:], acc0[:], ej[:, col])
                        elif c in GP_H1:
                            if c == GP_H1[0]:
                                nc.gpsimd.tensor_copy(accg[:], ej[:, col])
                            else:
                                nc.gpsimd.tensor_add(accg[:], accg[:], ej[:, col])
                        else:
                            if sl == 1:
                                nc.vector.tensor_copy(acc[:], ej[:, col])
                            else:
                                nc.vector.tensor_add(acc[:], acc[:], ej[:, col])
                if k >= 2:
                    ejp, slsp = e_hist.pop(k - 2)
                    for i, sl in enumerate(slsp):
                        c, hh = divmod(sl, 2)
                        nc.tensor.matmul(
                            out_ps[:, 512 * hh : 512 * (hh + 1)],
                            vt_bf[:, 128 * c : 128 * (c + 1)],
                            ejp[:, 512 * i : 512 * (i + 1)],
                            start=(sl <= 1),
                            stop=(sl >= 2 * NCHUNK - 2),
                        )

            # cross-partition folds of the three accumulators (into stream
            # banks freed by the finished exp units)
            s_ps = stage.tile([128, 512], F32, tag="S")
            nc.tensor.matmul(
                s_ps[0:1, 0:512], ones_col[:], acc0[:], start=True, stop=True
            )
            sfold = lpb.tile([128, MBLK], F32, tag="Lh")
            nc.tensor.matmul(
                sfold[0:1, 0:512], ones_col[:], acc[:], start=True, stop=False
            )
            nc.tensor.matmul(
                sfold[0:1, 0:512], ones_col[:], accg[:], start=False, stop=True
            )

            # tail in 512-wide halves: r = 1/s via the fast DVE reciprocal
            # (fp32, ~18 bits, reads s straight from PSUM), broadcast across
            # partitions with a K=1 PE matmul into a freed lpool bank and a
            # ScalarE copy back to SBUF (PE and ACT are both idle here, and
            # DVE may read only one PSUM operand), then a DVE multiply and
            # one fused multiply-add: out = (outPV * r) * gamma + zc' with
            # zc' = zc + gamma*bv folded on the host.  No exp/ln involved,
            # and gamma = 0 / gamma < 0 need no special casing.
            for h in range(2):
                sl = slice(h * 512, (h + 1) * 512)
                s_src = s_ps[0:1, 0:512] if h == 0 else sfold[0:1, 0:512]
                nc.vector.reciprocal_approx_fast(out=rvec[0:1, sl], in_=s_src)
                nc.gpsimd.partition_broadcast(rb_sb[:, sl], rvec[0:1, sl])
                nc.vector.tensor_tensor(
                    tmp_sb[:, sl], out_ps[:, sl], rb_sb[:, sl], op=ALU.mult
                )
                nc.vector.scalar_tensor_tensor(
                    out_sb[:, sl],
                    tmp_sb[:, sl],
                    gam_ap,
                    zc_sb[:, sl],
                    op0=ALU.mult,
                    op1=ALU.add,
                )
                # separate queues so the two output transfers overlap;
                # ScalarE is idle at the tail, GPSIMD is doing broadcasts
                eng = nc.sync if h == 0 else nc.scalar
                eng.dma_start(out_d[:, sl], out_sb[:, sl])

    nc.compile()
    return nc


_CACHE = {}


def _get_program(use_qk_bias: bool):
    if use_qk_bias not in _CACHE:
        _CACHE[use_qk_bias] = _build(use_qk_bias)
    return _CACHE[use_qk_bias]


def kernel(zc, zm, Wq, bq, Wk, bk, Wv, bv, gamma):
    global LAST_RESULTS
    zc = np.ascontiguousarray(zc, dtype=np.float32)
    zmf = np.asarray(zm, dtype=np.float32).reshape(B, CM, N)
    zmf_bf = zmf.astype(ml_dtypes.bfloat16)
    zcf = zc.reshape(B, CC, N)

    Wq = np.asarray(Wq, dtype=np.float32)
    Wk = np.asarray(Wk, dtype=np.float32)
    Wv = np.asarray(Wv, dtype=np.float32)
    # lps[n,m] = sum_c zm[c,n] tq[c,m] must equal k_n . q_m = zm_n^T (Wk^T Wq) zm_m,
    # so tq = (Wk^T Wq) zm_q; the tq matmul computes gt^T @ zm_q, hence
    # gt = (Wk^T Wq)^T = Wq^T Wk.
    gt = (Wq.astype(np.float64).T @ Wk.astype(np.float64)).astype(
        ml_dtypes.bfloat16
    )
    wvt = np.ascontiguousarray(Wv.T).astype(ml_dtypes.bfloat16)
    gamma_v = np.float32(np.asarray(gamma).reshape(-1)[0])
    sc_arr = np.full((CC, 1), gamma_v, dtype=np.float32)
    # zc' = zc + gamma*bv, so the kernel tail is a single multiply-add
    zcf = zcf + (gamma_v * np.asarray(bv, dtype=np.float32))[None, :, None]

    use_qk_bias = bool(np.any(bq)) or bool(np.any(bk))
    nc = _get_program(use_qk_bias)

    in_maps = []
    for c in range(NCORES):
        b, jblk = divmod(c, 4)
        m = {
            "zm": np.ascontiguousarray(
                np.roll(zmf_bf[b], -MBLK * jblk, axis=1)
            ),
            "zc": np.ascontiguousarray(zcf[b][:, MBLK * jblk : MBLK * (jblk + 1)]),
            "gt": gt,
            "wvt": wvt,
            "sc": sc_arr,
            "onesc": np.ones((128, 1), dtype=ml_dtypes.bfloat16),
        }
        if use_qk_bias:
            m["u"] = np.ascontiguousarray(
                (Wk.T @ np.asarray(bq, dtype=np.float32)).reshape(CM, 1)
            ).astype(ml_dtypes.bfloat16)
        in_maps.append(m)

    trace = bool(int(os.environ.get("BASS_KERNEL_TRACE", "0")))
    if trace and not _ensure_ntff_hook():
        trace = False
    res = run_bass_kernel_spmd(
        nc,
        in_maps,
        core_ids=list(range(NCORES)),
        trace=trace,
    )
    LAST_RESULTS = res

    out = np.empty((B, CC, N), dtype=np.float32)
    for c in range(NCORES):
        b, jblk = divmod(c, 4)
        out[b][:, MBLK * jblk : MBLK * (jblk + 1)] = res.results[c]["out"]
    return out.reshape(zc.shape)


# revision 76
# speedup vs baseline: 1.2682x; 1.0069x over previous
"""Trainium2 Bass kernel for nn_AttentionAggregator3d.

Math (per batch b):
    zmf = zm.reshape(CM, N)                     # N = D*W*H = 4096 tokens
    q = Wq @ zmf + bq ; k = Wk @ zmf + bk       # (16, N)
    v = Wv @ zmf + bv                           # (128, N)
    A = softmax_n(q^T k)                        # (N, N), softmax over keys n
    out = v @ A^T ; result = zc + gamma * out

Kernel structure (ScalarE-stream-bound design, ~58-60us/core):
  * logits^T[n, m] = k_n . q_m with G = Wk^T Wq folded on host; the
    query-side transform tq = G^T zm_q (128 x 1024) is computed ONCE and
    used as the bf16 MOVING operand of every logits matmul, with the key
    chunk zm_j as the stationary.  (The old version materialized t = G zm
    over all 4096 keys and burned 8 ScalarE copies staging it; ScalarE is
    the bottleneck engine, so those copies came straight out of the
    critical path.)
  * All matmul operands are bf16 (host pre-converts zm/G/Wv^T); exp output
    E is bf16 too, which halves SBUF traffic and lets the DVE denominator
    accumulation run in the 16-bit DVE perf modes.
  * Steady state: ScalarE streams 32 back-to-back [128,1024] exps
    (~1.05us each = the roofline); TensorE (~95% busy) does per chunk two
    512-wide logits matmuls, two 512-wide PV accumulations and an
    amortized 1/4 vproj batch; DVE accumulates both denominator halves
    except 8 early-chunk halves on GPSIMD.
  * Tail: three ones-matmul folds -> fast DVE reciprocal (straight from
    PSUM) -> GPSIMD partition_broadcast -> DVE multiply + fused
    multiply-add (gamma and gamma*bv are folded host-side into a single
    scale and into zc, so gamma=0 / negative gamma need no special path).
  * Sharding: 8 cores = batch (2) x query-block (4, 1024 queries each),
    zm rolled per core so its query block sits at columns 0:1024.
  * ACT tables are prefetched with a dummy exp at t=0 so the table load
    overlaps the input DMAs; the zm[:, 0:1024] DMA (which gates the first
    exp) is split across two queues because transfers serialize per queue.
"""

import os
import sys
import types

import ml_dtypes
import numpy as np

import concourse.bacc as bacc_mod
import concourse.tile as tile
from concourse import mybir
from concourse.bass_utils import run_bass_kernel_spmd

B, CC, CM, P = 2, 128, 128, 16
N = 16 * 16 * 16          # 4096 tokens
MBLK = N // 4             # 1024 queries per core
NCORES = 8
NCHUNK = N // 128         # 32 key chunks of 128

F32 = mybir.dt.float32
F32R = mybir.dt.float32r
BF16 = mybir.dt.bfloat16
AF = mybir.ActivationFunctionType
ALU = mybir.AluOpType

LAST_RESULTS = None  # BassKernelResults of the most recent run (for test.py)


def _ensure_ntff_hook() -> bool:
    """The grading image lacks antenv.axon_hooks; synthesize it from the
    boot module's ctypes NTFF driver so trace=True works under axon."""
    try:
        import antenv.axon_hooks  # noqa: F401

        return True
    except ImportError:
        pass
    try:
        import antenv
        from trn_agent_boot.trn_boot import _ntff_profile_via_ctypes

        hook = _ntff_profile_via_ctypes("/opt/axon/libaxon_pjrt.so")
        mod = types.ModuleType("antenv.axon_hooks")
        mod.get_axon_ntff_profile_hook = lambda: hook
        mod.set_axon_ntff_profile_hook = lambda h: None
        sys.modules["antenv.axon_hooks"] = mod
        antenv.axon_hooks = mod
        return hook is not None
    except Exception:
        return False


# Route Exp and Ln to the one table set that holds both, so the kernel pays a
# single ACT_TABLE_LOAD instead of three (exp -> ln -> exp again).
_orig_gat = bacc_mod.get_activation_tables
_COMBINED_SET = "natural_log_exp_and_others"


def _patched_gat(arch):
    tabs = _orig_gat(arch)
    if _COMBINED_SET in tabs:
        for name, fns in tabs.items():
            if name != _COMBINED_SET:
                fns.discard(AF.Exp)
                fns.discard(AF.Ln)
    return tabs


bacc_mod.get_activation_tables = _patched_gat


def _build(use_qk_bias: bool):
    nc = bacc_mod.Bacc(
        "TRN2",
        target_bir_lowering=False,
        debug=False,
        num_devices=NCORES,
    )

    zm_d = nc.dram_tensor("zm", (CM, N), BF16, kind="ExternalInput").ap()
    zc_d = nc.dram_tensor("zc", (CC, MBLK), F32, kind="ExternalInput").ap()
    gt_d = nc.dram_tensor("gt", (CM, CM), BF16, kind="ExternalInput").ap()
    wvt_d = nc.dram_tensor("wvt", (CM, CC), BF16, kind="ExternalInput").ap()
    # packed per-partition scalars: col 0 = gamma (gamma*bv is pre-added
    # into zc on the host)
    sc_d = nc.dram_tensor("sc", (CC, 1), F32, kind="ExternalInput").ap()
    onesc_d = nc.dram_tensor("onesc", (128, 1), BF16, kind="ExternalInput").ap()
    if use_qk_bias:
        u_d = nc.dram_tensor("u", (CM, 1), BF16, kind="ExternalInput").ap()
    out_d = nc.dram_tensor("out", (CC, MBLK), F32, kind="ExternalOutput").ap()

    # denominator routing (see module docstring): half 0 accumulates on the
    # DVE (acc0); half 1 goes to GPSIMD for early chunks (it is slow per
    # add, so keep it off the final-chunk critical path), else to the DVE
    # acc accumulator.  PE is the stream pacer at ~95%+ busy, so it gets no
    # ones-matmuls during the stream; the cross-partition folds happen in
    # three matmuls after the last exp.
    GP_H1 = [j for j in range(NCHUNK) if j % 3 == 1 and j < 24]

    with tile.TileContext(nc) as tc:
        with (
            tc.tile_pool(name="consts", bufs=1) as consts,
            tc.tile_pool(name="epool", bufs=6) as epool,
            tc.tile_pool(name="lpa", bufs=1, space="PSUM") as lpa,
            tc.tile_pool(name="lpb", bufs=1, space="PSUM") as lpb,
            tc.tile_pool(name="stage", bufs=1, space="PSUM") as stage,
            tc.tile_pool(name="opool", bufs=1, space="PSUM") as opool,
        ):
            zm_bf = consts.tile([CM, N], BF16, tag="zm")
            tq_bf = consts.tile([CM, MBLK], BF16, tag="tq")
            vt_bf = consts.tile([128, N], BF16, tag="vt")  # chunk j at cols 128j
            zc_sb = consts.tile([CC, MBLK], F32, tag="zc")
            gt_sb = consts.tile([CM, CM], BF16, tag="gt")
            wvt_sb = consts.tile([CM, CC], BF16, tag="wvt")
            sc_sb = consts.tile([CC, 1], F32, tag="sc")
            ones_col = consts.tile([128, 1], BF16, tag="onesc")
            acc0 = consts.tile([128, 512], BF16, tag="acc0")   # DVE, half 0
            acc = consts.tile([128, 512], BF16, tag="acc")     # DVE, half 1
            accg = consts.tile([128, 512], BF16, tag="accg")   # GPSIMD, half 1
            rvec = consts.tile([1, MBLK], F32, tag="rvec")
            rb_sb = consts.tile([128, MBLK], F32, tag="rb")
            tmp_sb = consts.tile([CC, MBLK], F32, tag="tmp")
            out_sb = consts.tile([CC, MBLK], F32, tag="outsb")
            scr = consts.tile([1, 1], F32, tag="scr")
            if use_qk_bias:
                u_sb = consts.tile([CM, 1], BF16, tag="u")
                rn_sb = consts.tile([128, NCHUNK], F32, tag="rn")

            # (A PE warm-up via junk matmuls during the DMA wait was tried
            # and removed: the HAM un-throttle lags the busy burst by ~2us,
            # so the dummies delay the first real matmul without making the
            # tq/logits-0 chain run warm.)

            # ---- input DMAs.  The critical chain is zm[:, 0:1024] + gt ->
            # tq -> cast -> logits 0 -> exp 0.  DMA *transfers* serialize per
            # issuing queue (~2.5us per 1024 bf16 columns), so the two tq
            # halves of zm go on different queues (sync / scalar) and the
            # rest follows on gpsimd, ordered by first use. ----
            nc.sync.dma_start(zm_bf[:, 0:512], zm_d[:, 0:512])
            nc.scalar.dma_start(gt_sb[:], gt_d)
            nc.scalar.dma_start(zm_bf[:, 512:1024], zm_d[:, 512:1024])

            # ACT table prefetch: a dummy exp with no real consumers makes
            # ScalarE pay the ~2.7us exp/ln table load during the input DMAs
            # instead of in front of the first real exp
            nc.scalar.activation(scr[0:1, 0:1], gt_sb[0:1, 0:1], AF.Exp)

            nc.sync.dma_start(ones_col[:], onesc_d)
            nc.gpsimd.dma_start(wvt_sb[:], wvt_d)
            if use_qk_bias:
                nc.gpsimd.dma_start(u_sb[:], u_d)
            nc.gpsimd.dma_start(zm_bf[:, 1024:2048], zm_d[:, 1024:2048])
            nc.gpsimd.dma_start(zm_bf[:, 2048:3072], zm_d[:, 2048:3072])
            nc.gpsimd.dma_start(zm_bf[:, 3072:4096], zm_d[:, 3072:4096])
            nc.gpsimd.dma_start(sc_sb[:], sc_d)
            nc.sync.dma_start(zc_sb[:], zc_d)

            gam_ap = sc_sb[:, 0:1]

            out_ps = opool.tile([CC, MBLK], F32, tag="out")

            # ---- tq = G zm_q over the core's 1024 query columns; cast in
            # halves so the DVE cast of half 0 overlaps the half-1 matmul ----
            tq_ps = lpa.tile([128, 1536], F32, tag="P")
            for h in range(2):
                nc.tensor.matmul(
                    tq_ps[:, h * 512 : (h + 1) * 512],
                    gt_sb[:],
                    zm_bf[:, h * 512 : (h + 1) * 512],
                    start=True,
                    stop=True,
                )
                nc.vector.tensor_copy(
                    tq_bf[:, h * 512 : (h + 1) * 512],
                    tq_ps[:, h * 512 : (h + 1) * 512],
                )

            def emit_vt_batch(i):
                # vt chunk j = (zm chunk j)^T @ Wv^T for j in 4i..4i+3
                vps = stage.tile([128, 512], F32, tag="S")
                for k in range(4):
                    j = 4 * i + k
                    nc.tensor.matmul(
                        vps[:, 128 * k : 128 * (k + 1)],
                        zm_bf[:, 128 * j : 128 * (j + 1)],
                        wvt_sb[:],
                        start=True,
                        stop=True,
                    )
                nc.vector.tensor_copy(vt_bf[:, i * 512 : (i + 1) * 512], vps[:])
                if use_qk_bias:
                    rnps = stage.tile([128, 512], F32, tag="S")
                    for k in range(4):
                        j = 4 * i + k
                        nc.tensor.matmul(
                            rnps[:, k : k + 1],
                            zm_bf[:, 128 * j : 128 * (j + 1)],
                            u_sb[:],
                            start=True,
                            stop=True,
                        )
                    nc.vector.tensor_copy(
                        rn_sb[:, 4 * i : 4 * (i + 1)], rnps[:, 0:4]
                    )

            if use_qk_bias:
                # the chunk-0..3 exp biases must be materialized before the
                # first exp reads them
                emit_vt_batch(0)

            # The exp stream is organized in UNITS of 512-wide half-chunk
            # slices (slice s = (chunk s//2, query-half s%2)).  Wider
            # ACTIVATEs amortize the ~185ns per-instruction init, but PSUM
            # allows only one 3-bank tile next to the 2-bank one (lpa+lpb+
            # stage+opool = 8 banks), so units alternate 1024/1536 wide.
            # Both unit pools are single-buffered; overlap comes from
            # alternating between them, and PV runs at a lag of TWO units so
            # the next unit's logits matmuls always precede PV work on the
            # PE queue (otherwise the 3-matmul refill of the wide tile
            # cannot hide inside the narrow exp's window).
            # The qk-bias variant needs a per-chunk exp bias, so it uses
            # chunk-aligned 1024-wide units only.
            # (size, pool) per unit; pools must strictly alternate or two
            # consecutive units share a single-buffered tile and serialize
            # the exp stream.  The odd slice count is absorbed by hosting
            # one narrow unit in the wide pool right after tq.
            if use_qk_bias:
                spec = [(2, "b"), (2, "a")] * 16
            else:
                spec = [(2, "b"), (2, "a")] + [(2, "b"), (3, "a")] * 12
            units = []
            s0 = 0
            for sz, pl in spec:
                units.append((list(range(s0, s0 + sz)), pl))
                s0 += sz
            assert s0 == 2 * NCHUNK

            e_hist = {}
            for k in range(len(units) + 2):
                if k < len(units):
                    sls, pl = units[k]
                    w = 512 * len(sls)
                    if pl == "a":
                        lps = lpa.tile([128, 1536], F32, tag="P")
                    else:
                        lps = lpb.tile([128, MBLK], F32, tag="Lh")
                    for i, sl in enumerate(sls):
                        c, hh = divmod(sl, 2)
                        nc.tensor.matmul(
                            lps[:, 512 * i : 512 * (i + 1)],
                            zm_bf[:, 128 * c : 128 * (c + 1)],
                            tq_bf[:, 512 * hh : 512 * (hh + 1)],
                            start=True,
                            stop=True,
                        )
                    ej = epool.tile([128, w], BF16, tag=f"E{len(sls)}")
                    bias = (
                        rn_sb[:, sls[0] // 2 : sls[0] // 2 + 1]
                        if use_qk_bias
                        else 0.0
                    )
                    nc.scalar.activation(ej[:], lps[:, 0:w], AF.Exp, bias=bias)
                    e_hist[k] = (ej, sls)
                    if k == len(units) - 1:
                        last_e = (ej, sls)
                    # value-projection batches, triggered by chunk progress
                    for sl in sls:
                        c, hh = divmod(sl, 2)
                        if hh != 0:
                            continue
                        if c == 1 and not use_qk_bias:
                            emit_vt_batch(0)
                        elif c % 4 == 2 and c // 4 + 1 <= 7:
                            emit_vt_batch(c // 4 + 1)
                    # softmax-denominator accumulation.  The FINAL unit is
                    # excluded: its E slices fold directly into the PE
                    # ones-matmul groups below, so the tail does not wait
                    # for a last round of DVE adds after the last exp.
                    for i, sl in enumerate(sls if k < len(units) - 1 else []):
                        col = slice(512 * i, 512 * (i + 1))
                        c, hh = divmod(sl, 2)
                        if hh == 0:
                            if sl == 0:
                                nc.vector.tensor_copy(acc0[:], ej[:, col])
                            else:
                                nc.vector.tensor_add(acc0[:], acc0[:], ej[:, col])
                        elif c in GP_H1:
                            if c == GP_H1[0]:
                                nc.gpsimd.tensor_copy(accg[:], ej[:, col])
                            else:
                                nc.gpsimd.tensor_add(accg[:], accg[:], ej[:, col])
                        else:
                            if sl == 1:
                                nc.vector.tensor_copy(acc[:], ej[:, col])
                            else:
                                nc.vector.tensor_add(acc[:], acc[:], ej[:, col])
                if k >= 2:
                    ejp, slsp = e_hist.pop(k - 2)
                    for i, sl in enumerate(slsp):
                        c, hh = divmod(sl, 2)
                        nc.tensor.matmul(
                            out_ps[:, 512 * hh : 512 * (hh + 1)],
                            vt_bf[:, 128 * c : 128 * (c + 1)],
                            ejp[:, 512 * i : 512 * (i + 1)],
                            start=(sl <= 1),
                            stop=(sl >= 2 * NCHUNK - 2),
                        )

            # cross-partition folds of the three accumulators (into stream
            # banks freed by the finished exp units), with the final unit's
            # E slices folded in directly
            ej_l, sls_l = last_e
            h0_ex = [i for i, sl in enumerate(sls_l) if sl % 2 == 0]
            h1_ex = [i for i, sl in enumerate(sls_l) if sl % 2 == 1]
            s_ps = stage.tile([128, 512], F32, tag="S")
            nc.tensor.matmul(
                s_ps[0:1, 0:512], ones_col[:], acc0[:],
                start=True, stop=(not h0_ex),
            )
            for n, i in enumerate(h0_ex):
                nc.tensor.matmul(
                    s_ps[0:1, 0:512], ones_col[:],
                    ej_l[:, 512 * i : 512 * (i + 1)],
                    start=False, stop=(n == len(h0_ex) - 1),
                )
            sfold = lpb.tile([128, MBLK], F32, tag="Lh")
            nc.tensor.matmul(
                sfold[0:1, 0:512], ones_col[:], acc[:], start=True, stop=False
            )
            nc.tensor.matmul(
                sfold[0:1, 0:512], ones_col[:], accg[:],
                start=False, stop=(not h1_ex),
            )
            for n, i in enumerate(h1_ex):
                nc.tensor.matmul(
                    sfold[0:1, 0:512], ones_col[:],
                    ej_l[:, 512 * i : 512 * (i + 1)],
                    start=False, stop=(n == len(h1_ex) - 1),
                )

            # tail in 512-wide halves: r = 1/s via the fast DVE reciprocal
            # (fp32, ~18 bits, reads s straight from PSUM), broadcast across
            # partitions with a K=1 PE matmul into a freed lpool bank and a
            # ScalarE copy back to SBUF (PE and ACT are both idle here, and
            # DVE may read only one PSUM operand), then a DVE multiply and
            # one fused multiply-add: out = (outPV * r) * gamma + zc' with
            # zc' = zc + gamma*bv folded on the host.  No exp/ln involved,
            # and gamma = 0 / gamma < 0 need no special casing.
            for h in range(2):
                sl = slice(h * 512, (h + 1) * 512)
                s_src = s_ps[0:1, 0:512] if h == 0 else sfold[0:1, 0:512]
                nc.vector.reciprocal_approx_fast(out=rvec[0:1, sl], in_=s_src)
                nc.gpsimd.partition_broadcast(rb_sb[:, sl], rvec[0:1, sl])
                nc.vector.tensor_tensor(
                    tmp_sb[:, sl], out_ps[:, sl], rb_sb[:, sl], op=ALU.mult
                )
                nc.vector.scalar_tensor_tensor(
                    out_sb[:, sl],
                    tmp_sb[:, sl],
                    gam_ap,
                    zc_sb[:, sl],
                    op0=ALU.mult,
                    op1=ALU.add,
                )
                # separate queues so the two output transfers overlap;
                # ScalarE is idle at the tail, GPSIMD is doing broadcasts
                eng = nc.sync if h == 0 else nc.scalar
                eng.dma_start(out_d[:, sl], out_sb[:, sl])

    nc.compile()
    return nc


_CACHE = {}


def _get_program(use_qk_bias: bool):
    if use_qk_bias not in _CACHE:
        _CACHE[use_qk_bias] = _build(use_qk_bias)
    return _CACHE[use_qk_bias]


def kernel(zc, zm, Wq, bq, Wk, bk, Wv, bv, gamma):
    global LAST_RESULTS
    zc = np.ascontiguousarray(zc, dtype=np.float32)
    zmf = np.asarray(zm, dtype=np.float32).reshape(B, CM, N)
    zmf_bf = zmf.astype(ml_dtypes.bfloat16)
    zcf = zc.reshape(B, CC, N)

    Wq = np.asarray(Wq, dtype=np.float32)
    Wk = np.asarray(Wk, dtype=np.float32)
    Wv = np.asarray(Wv, dtype=np.float32)
    # lps[n,m] = sum_c zm[c,n] tq[c,m] must equal k_n . q_m = zm_n^T (Wk^T Wq) zm_m,
    # so tq = (Wk^T Wq) zm_q; the tq matmul computes gt^T @ zm_q, hence
    # gt = (Wk^T Wq)^T = Wq^T Wk.
    gt = (Wq.astype(np.float64).T @ Wk.astype(np.float64)).astype(
        ml_dtypes.bfloat16
    )
    wvt = np.ascontiguousarray(Wv.T).astype(ml_dtypes.bfloat16)
    gamma_v = np.float32(np.asarray(gamma).reshape(-1)[0])
    sc_arr = np.full((CC, 1), gamma_v, dtype=np.float32)
    # zc' = zc + gamma*bv, so the kernel tail is a single multiply-add
    zcf = zcf + (gamma_v * np.asarray(bv, dtype=np.float32))[None, :, None]

    use_qk_bias = bool(np.any(bq)) or bool(np.any(bk))
    nc = _get_program(use_qk_bias)

    in_maps = []
    for c in range(NCORES):
        b, jblk = divmod(c, 4)
        m = {
            "zm": np.ascontiguousarray(
                np.roll(zmf_bf[b], -MBLK * jblk, axis=1)
            ),
            "zc": np.ascontiguousarray(zcf[b][:, MBLK * jblk : MBLK * (jblk + 1)]),
            "gt": gt,
            "wvt": wvt,
            "sc": sc_arr,
            "onesc": np.ones((128, 1), dtype=ml_dtypes.bfloat16),
        }
        if use_qk_bias:
            m["u"] = np.ascontiguousarray(
                (Wk.T @ np.asarray(bq, dtype=np.float32)).reshape(CM, 1)
            ).astype(ml_dtypes.bfloat16)
        in_maps.append(m)

    trace = bool(int(os.environ.get("BASS_KERNEL_TRACE", "0")))
    if trace and not _ensure_ntff_hook():
        trace = False
    res = run_bass_kernel_spmd(
        nc,
        in_maps,
        core_ids=list(range(NCORES)),
        trace=trace,
    )
    LAST_RESULTS = res

    out = np.empty((B, CC, N), dtype=np.float32)
    for c in range(NCORES):
        b, jblk = divmod(c, 4)
        out[b][:, MBLK * jblk : MBLK * (jblk + 1)] = res.results[c]["out"]
    return out.reshape(zc.shape)
